# revision 1
# baseline (speedup 1.0000x reference)
"""Trainium2 Bass kernel for nn_Attention_31997506355363 (sparse_attention).

Sharding: 8 cores = 2 batches x 4 head-groups (4 heads of 16 each).
Each core computes its batch's full-sequence double-attend for its 4 heads,
plus the partial output projection (Wout rows for its heads); host sums the
4 head-group partials per batch.

Math notes (verified vs reference in fp64 to ~9e-7 rel):
  - mask keeps j<=i OR j>i+512  (the strip i<j<=i+512 is masked out)
  - softmax has a per-head sink logit in the denominator only
  - |sim| <= ~6.4 so softmax runs without max-subtraction: p = exp(sim),
    denom = sum_j p + exp(sink)
  - attends are computed transposed: simT[j,i] tiles -> exp -> outT
    accumulated as v.T @ p per 128-j-block (contraction always on the
    partition dim, so no attention-matrix transposes are needed, and
    attend1's output hiddensT feeds attend2 directly)
  - projection outputs bounce through DRAM; the attend working set is
    streamed back per (head, pass)
"""

import sys

for _p in ("/opt/trn_rl_repo",):
    if _p not in sys.path:
        sys.path.insert(0, _p)

import numpy as np
import concourse.bass as bass
from concourse import bacc
import concourse.mybir as mybir
from concourse.tile import TileContext
from concourse.vector_clock import ScopedClock
from concourse.masks import make_identity
import bass_rust

FP32 = mybir.dt.float32
N_CORES = 8
N = 2048            # sequence length
DQ = 1024           # model dim
HEADS = 4           # heads per core
SCALE = 0.125       # 64 ** -0.5, folded into k1T / k2T at projection copy
NB = N // 128       # 16 key blocks
PASS = 1024         # attend i-pass width (2 passes)
ACT = mybir.ActivationFunctionType

# matmul input dtype.  float32r looks 4x faster in the cost model but its
# fused 4-byte weight self-load measures ~150us per matmul on this HW
# (~250ms/body vs ~2ms with plain float32), so float32 wins decisively and
# is also bit-accurate.
MM_DT = mybir.dt.float32
DEBUG = False
REPS = 1
SKIP_GPSIMD = False  # timing experiment: drop gpsimd ops in attends (wrong results)
PROJ_ONLY = False    # timing experiment: stop after projections            # kernel-body repetitions (timing only; leave 1 for grading)


class PatchedTileContext(TileContext):
    """This walrus build rejects >1 sync-wait on the tail Drain; split the
    tail-drain waits across multiple unfusable drain instructions."""

    def _drain_and_barrier(self, tick_clock, wait_clock):
        drain_inst = self.nc.sync.drain(fusable=False)
        wait_clock.add_sem_waits(
            drain_inst.ins, ScopedClock({None: tick_clock.global_clock})
        )
        waits = list(drain_inst.ins.sync_info.on_wait or [])
        if len(waits) > 1:
            drain_inst.ins.sync_info.on_wait = waits[:1]
            for i in range(1, len(waits)):
                d2 = self.nc.sync.drain(fusable=False)
                d2.ins.sync_info = bass_rust.SyncInfo(
                    on_wait=waits[i:i + 1], on_update=[]
                )
        self.nc.all_engine_barrier()
        popped = self.nc._tile_sem_poison_stack.pop()
        assert popped is self._sem_poison
        self.nc.clear_and_free_semaphores(list(self.sems.allocated().values()))
        self.nc.all_engine_barrier()


def _bank_chunks(col, w):
    """Split [col, col+w) at 512-column PSUM bank boundaries (a matmul
    output must stay within one 2KB bank)."""
    out = []
    while w > 0:
        take = min(w, 512 - (col % 512))
        out.append((col, take))
        col += take
        w -= take
    return out


def _runs_for(jb, p):
    """i-subblock runs (in 128-col units within a 1024-wide pass) that are
    not fully masked for key-block jb.  Sub-block t covers queries
    I = 8p + t; (I, jb) is fully masked iff 1 <= jb - I <= 3."""
    skip_lo = max(0, jb - 8 * p - 3)
    skip_hi = min(8, jb - 8 * p)
    if skip_lo >= skip_hi:
        return [(0, 8)], None
    runs = []
    if skip_lo > 0:
        runs.append((0, skip_lo))
    if skip_hi < 8:
        runs.append((skip_hi, 8))
    return runs, (skip_lo, skip_hi)


def build_kernel(nc, tc, io):
    mm = nc.tensor.matmul

    def fill_fr(ap, val, width):
        # memset is not ISA-legal for float32r; affine_select with an
        # always-false predicate fills unconditionally
        nc.gpsimd.affine_select(
            out=ap, in_=ap, compare_op=mybir.AluOpType.is_ge, fill=val,
            base=-1, pattern=[[0, width]], channel_multiplier=0)

    def mmr(out, lhsT, rhs, start, stop):
        mm(out, lhsT, rhs, start=start, stop=stop)

    xq, xkv = io["xq"], io["xkv"]
    wq, wk1, wv1, wk2, wv2, wout, sink = (
        io["wq"], io["wk1"], io["wv1"], io["wk2"], io["wv2"], io["wout"],
        io["sink"],
    )
    out = io["out"]

    const = tc.alloc_tile_pool(name="const", bufs=1)
    stat = tc.alloc_tile_pool(name="stat", bufs=1)
    xin = tc.alloc_tile_pool(name="xin", bufs=1)
    xtp = tc.alloc_tile_pool(name="xt", bufs=1)
    wpool = tc.alloc_tile_pool(name="w", bufs=10)
    stg = tc.alloc_tile_pool(name="stg", bufs=3)
    kst = tc.alloc_tile_pool(name="kst", bufs=2)
    vst = tc.alloc_tile_pool(name="vst", bufs=4)
    epool = tc.alloc_tile_pool(name="e", bufs=3)
    npool = tc.alloc_tile_pool(name="nrm", bufs=2)
    osb_p = tc.alloc_tile_pool(name="osb", bufs=2)
    dram = tc.alloc_tile_pool(name="dram", bufs=1, space="DRAM")
    ps_sim = tc.alloc_tile_pool(name="ps_sim", bufs=2, space="PSUM")
    ps_av = tc.alloc_tile_pool(name="ps_av", bufs=1, space="PSUM")
    ps_ones = tc.alloc_tile_pool(name="ps_ones", bufs=1, space="PSUM")
    _pools = [const, stat, xin, xtp, wpool, stg, kst, vst, epool, npool,
              osb_p, dram, ps_sim, ps_av, ps_ones]

    # ---- constants ----
    ident = const.tile([128, 128], FP32, tag="ident", name="ident")
    make_identity(nc, ident[:])
    onescol = const.tile([128, 1], MM_DT, tag="onescol", name="onescol")
    fill_fr(onescol[:], 1.0, 1)

    sink_sb = const.tile([1, HEADS], FP32, tag="sink", name="sink")
    nc.sync.dma_start(out=sink_sb[:], in_=sink[:])
    esink = const.tile([1, HEADS], FP32, tag="esink", name="esink")
    nc.scalar.activation(esink[:], sink_sb[:], ACT.Exp)
    sinkb = const.tile([128, HEADS], FP32, tag="sinkb", name="sinkb")
    nc.gpsimd.partition_broadcast(sinkb[:], esink[0:1, :])
    ones4 = const.tile([128, HEADS], FP32, tag="ones4", name="ones4")
    nc.gpsimd.memset(ones4[:], 1.0)

    # ---- SBUF statics ----
    o2T = [stat.tile([128, N], MM_DT, tag=f"o2T{t}", name=f"o2T{t}") for t in range(2)]
    wout_sb = [stat.tile([128, DQ], MM_DT, tag=f"wo{t}", name=f"wo{t}") for t in range(2)]
    for t in range(2):
        nc.sync.dma_start(out=wout_sb[t][:], in_=wout[t * 128:(t + 1) * 128, :])

    # ---- DRAM intermediates ----
    qT_d = dram.tile([256, N], MM_DT, tag="qT_d", name="qT_d")
    k1T_d = dram.tile([256, N], MM_DT, tag="k1T_d", name="k1T_d")
    k2T_d = dram.tile([512, N], MM_DT, tag="k2T_d", name="k2T_d")
    v1_d = dram.tile([N, 512], MM_DT, tag="v1_d", name="v1_d")
    v2a_d = dram.tile([N, 65 * HEADS], MM_DT, tag="v2a_d", name="v2a_d")

    # =====================================================================
    # Phase 0+1: per 512-wide n-chunk: transpose x, run projections,
    # bounce results to DRAM.
    # =====================================================================
    def transpose_chunk(x_nat):
        """x_nat: 4 tiles [128, 1024] -> 8 kt tiles [128(dim), 512(n)]."""
        res = []
        for kt in range(8):
            ps = ps_sim.tile([128, PASS], FP32, tag="sim", name="sim")
            for nbl in range(4):
                nc.tensor.transpose(
                    ps[:, nbl * 128:(nbl + 1) * 128],
                    x_nat[nbl][:, kt * 128:(kt + 1) * 128], ident[:])
            t = xtp.tile([128, 512], MM_DT, tag=f"xt{kt}", name=f"xt{kt}")
            if kt % 2 == 0:
                nc.vector.tensor_copy(t[:], ps[:, 0:512])
            else:
                nc.scalar.copy(t[:], ps[:, 0:512])
            res.append(t)
        return res

    def load_w(w_dram, cols):
        wt = [wpool.tile([128, cols], MM_DT, tag="w", name="w") for _ in range(8)]
        for kt in range(8):
            nc.sync.dma_start(out=wt[kt][:], in_=w_dram[kt * 128:(kt + 1) * 128, :])
        return wt

    for c in range(4):                    # n-chunks of 512
        ccols = slice(c * 512, (c + 1) * 512)

        # -- xq: transpose + qT projection --
        xq_nat = []
        for nbl in range(4):
            r0 = c * 512 + nbl * 128
            t1 = xin.tile([128, DQ], FP32, tag=f"xn{nbl}", name=f"xn{nbl}")
            nc.sync.dma_start(out=t1[:], in_=xq[r0:r0 + 128, :])
            xq_nat.append(t1)
        xqT = transpose_chunk(xq_nat)

        wt = load_w(wq, 256)
        for m in range(2):
            acc = ps_sim.tile([128, PASS], FP32, tag="sim", name="sim")
            for kt in range(8):
                mmr(acc[:, 0:512], wt[kt][:, m * 128:(m + 1) * 128], xqT[kt][:],
                    start=(kt == 0), stop=(kt == 7))
            s = stg.tile([128, 512], MM_DT, tag="stg", name="stg")
            nc.vector.tensor_copy(s[:], acc[:, 0:512])
            nc.sync.dma_start(out=qT_d[m * 128:(m + 1) * 128, ccols], in_=s[:])

        # -- xkv: transpose + k1/k2/v1/v2 projections --
        xkv_nat = []
        for nbl in range(4):
            r0 = c * 512 + nbl * 128
            t2 = xin.tile([128, DQ], FP32, tag=f"xn{nbl}", name=f"xn{nbl}")
            nc.sync.dma_start(out=t2[:], in_=xkv[r0:r0 + 128, :])
            xkv_nat.append(t2)
        xkvT = transpose_chunk(xkv_nat)

        wt = load_w(wk1, 256)
        for m in range(2):
            acc = ps_sim.tile([128, PASS], FP32, tag="sim", name="sim")
            for kt in range(8):
                mmr(acc[:, 0:512], wt[kt][:, m * 128:(m + 1) * 128], xkvT[kt][:],
                    start=(kt == 0), stop=(kt == 7))
            s = stg.tile([128, 512], MM_DT, tag="stg", name="stg")
            nc.scalar.mul(s[:], acc[:, 0:512], SCALE)
            nc.sync.dma_start(out=k1T_d[m * 128:(m + 1) * 128, ccols], in_=s[:])

        wt = load_w(wk2, 512)
        for m in range(4):
            acc = ps_sim.tile([128, PASS], FP32, tag="sim", name="sim")
            for kt in range(8):
                mmr(acc[:, 0:512], wt[kt][:, m * 128:(m + 1) * 128], xkvT[kt][:],
                    start=(kt == 0), stop=(kt == 7))
            s = stg.tile([128, 512], MM_DT, tag="stg", name="stg")
            nc.scalar.mul(s[:], acc[:, 0:512], SCALE)
            nc.sync.dma_start(out=k2T_d[m * 128:(m + 1) * 128, ccols], in_=s[:])

        wt = load_w(wv1, 512)
        for nbl in range(4):
            acc = ps_sim.tile([128, PASS], FP32, tag="sim", name="sim")
            for kt in range(8):
                mmr(acc[:, 0:512], xkvT[kt][:, nbl * 128:(nbl + 1) * 128], wt[kt][:],
                    start=(kt == 0), stop=(kt == 7))
            s = stg.tile([128, 512], MM_DT, tag="stg", name="stg")
            nc.vector.tensor_copy(s[:], acc[:, 0:512])
            r0 = c * 512 + nbl * 128
            nc.sync.dma_start(out=v1_d[r0:r0 + 128, :], in_=s[:])

        wt = load_w(wv2, 256)
        for nbl in range(4):
            acc = ps_sim.tile([128, PASS], FP32, tag="sim", name="sim")
            for kt in range(8):
                mmr(acc[:, 0:256], xkvT[kt][:, nbl * 128:(nbl + 1) * 128], wt[kt][:],
                    start=(kt == 0), stop=(kt == 7))
            s = stg.tile([128, 512], MM_DT, tag="stg", name="stg")
            # pack [h*64 cols] into 65-col groups with a ones column
            sv = s[:, 0:260].rearrange("p (h c) -> p h c", h=HEADS)
            nc.vector.tensor_copy(
                sv[:, :, 0:64],
                acc[:, 0:256].rearrange("p (h c) -> p h c", h=HEADS))
            nc.vector.tensor_copy(
                sv[:, :, 64:65],
                ones4[:].rearrange("p (h c) -> p h c", h=HEADS))
            r0 = c * 512 + nbl * 128
            nc.sync.dma_start(out=v2a_d[r0:r0 + 128, :], in_=s[:, 0:260])

    if DEBUG:
        for nm, t_ in (("dbg_qT", qT_d), ("dbg_k1T", k1T_d), ("dbg_k2T", k2T_d),
                       ("dbg_v1", v1_d), ("dbg_v2a", v2a_d)):
            nc.sync.dma_start(out=io[nm].bitcast(MM_DT), in_=t_[:, :])

    def dbg_sbuf(nm, ap):
        if DEBUG and nm in io:
            nc.sync.dma_start(out=io[nm].bitcast(ap.dtype), in_=ap)

    def dbg_psum(nm, ap, rows, cols):
        if DEBUG and nm in io:
            tmp = npool.tile([rows, cols], FP32, tag="dbgt", name="dbgt")
            nc.vector.tensor_copy(tmp[:], ap)
            nc.sync.dma_start(out=io[nm], in_=tmp[:])

    if PROJ_ONLY:
        # write something to out and stop
        for nb in range(NB):
            s0 = stg.tile([128, 512], MM_DT, tag="stg", name="stg")
            nc.sync.dma_start(out=s0[:], in_=v1_d[nb * 128:(nb + 1) * 128, :])
            nc.sync.dma_start(out=out[nb * 128:(nb + 1) * 128, 0:512].bitcast(MM_DT), in_=s0[:])
        for p_ in reversed(_pools):
            p_.release()
        return

    # =====================================================================
    # Phase 2: attends (streaming q/k/v slices back from DRAM)
    # =====================================================================
    def masked_exp_av(k_h, rhs_h, v_tiles, vcols, out_ps, ones_ps, p):
        """One attend pass: for each key block jb, sim -> exp -> mask ->
        accumulate v.T @ e (and optionally the ones row)."""
        for jb in range(NB):
            simp = ps_sim.tile([128, PASS], FP32, tag="sim", name="sim")
            runs, skip = _runs_for(jb, p)
            e = epool.tile([128, PASS], MM_DT, tag="e", name="e")
            for (t0, t1) in runs:
                for (col, w) in _bank_chunks(t0 * 128, (t1 - t0) * 128):
                    mmr(simp[:, col:col + w],
                        k_h[:, jb * 128:(jb + 1) * 128],
                        rhs_h[:, col:col + w],
                        start=True, stop=True)
                nc.scalar.activation(
                    e[:, t0 * 128:t1 * 128], simp[:, t0 * 128:t1 * 128],
                    ACT.Exp)
            if skip is not None and not SKIP_GPSIMD:
                fill_fr(e[:, skip[0] * 128:skip[1] * 128], 0.0,
                        (skip[1] - skip[0]) * 128)
            td = jb - 8 * p
            if SKIP_GPSIMD:
                td = -99
            if 0 <= td < 8:   # diagonal block: keep jj <= ii
                nc.gpsimd.affine_select(
                    out=e[:, td * 128:(td + 1) * 128],
                    in_=e[:, td * 128:(td + 1) * 128],
                    compare_op=mybir.AluOpType.is_ge, fill=0.0, base=0,
                    pattern=[[1, 128]], channel_multiplier=-1)
            ta = -99 if SKIP_GPSIMD else (jb - 4 - 8 * p)
            if 0 <= ta < 8:   # jb == I+4 block: keep jj > ii
                nc.gpsimd.affine_select(
                    out=e[:, ta * 128:(ta + 1) * 128],
                    in_=e[:, ta * 128:(ta + 1) * 128],
                    compare_op=mybir.AluOpType.is_ge, fill=0.0, base=-1,
                    pattern=[[-1, 128]], channel_multiplier=1)
            for s in range(2):
                mmr(out_ps[:, s * 512:(s + 1) * 512],
                    v_tiles[jb][:, vcols.start:vcols.stop],
                    e[:, s * 512:(s + 1) * 512],
                    start=(jb == 0), stop=(jb == NB - 1))
                if ones_ps is not None:
                    mmr(ones_ps[s][:], onescol[:],
                        e[:, s * 512:(s + 1) * 512],
                        start=(jb == 0), stop=(jb == NB - 1))

    for h in range(HEADS):
        k1h = kst.tile([64, N], MM_DT, tag="k1h", name="k1h")
        nc.sync.dma_start(out=k1h[:], in_=k1T_d[64 * h:64 * h + 64, :])
        k2h = kst.tile([128, N], MM_DT, tag="k2h", name="k2h")
        nc.sync.dma_start(out=k2h[:], in_=k2T_d[128 * h:128 * h + 128, :])
        for p in range(2):
            qh = kst.tile([64, PASS], MM_DT, tag="qh", name="qh")
            nc.sync.dma_start(out=qh[:], in_=qT_d[64 * h:64 * h + 64,
                                                  p * PASS:(p + 1) * PASS])
            v1s = []
            v2s = []
            for jb in range(NB):
                t = vst.tile([128, 128], MM_DT, tag="v1s", name="v1s")
                nc.sync.dma_start(
                    out=t[:], in_=v1_d[jb * 128:(jb + 1) * 128,
                                       128 * h:128 * h + 128])
                v1s.append(t)
                t = vst.tile([128, 65], MM_DT, tag="v2s", name="v2s")
                nc.sync.dma_start(
                    out=t[:], in_=v2a_d[jb * 128:(jb + 1) * 128,
                                        65 * h:65 * h + 65])
                v2s.append(t)

            # ------------- attend 1 -------------
            out1 = ps_av.tile([128, PASS], FP32, tag="av", name="av")
            ones = [ps_ones.tile([1, 512], FP32, tag=f"ones{s_}",
                                 name=f"ones{s_}") for s_ in range(2)]
            masked_exp_av(k1h, qh, v1s, slice(0, 128), out1, ones, p)

            if h == 0 and p == 0:
                dbg_psum("dbg_out1", out1[:], 128, PASS)
                dbg_psum("dbg_ones0", ones[0][:], 1, 512)
                dbg_psum("dbg_ones1", ones[1][:], 1, 512)

            # normalize + silu -> hT
            rb = npool.tile([128, PASS], FP32, tag="rb", name="rb")
            if SKIP_GPSIMD:
                nc.vector.memset(rb[:], 1.0)
            for s_ in range(SKIP_GPSIMD and 0 or 2):
                ds_ = npool.tile([1, 512], FP32, tag=f"ds{s_}",
                                 name=f"ds{s_}")
                nc.vector.tensor_copy(ds_[:], ones[s_][:])
                nc.vector.tensor_scalar_add(ds_[:], ds_[:],
                                            sinkb[0:1, h:h + 1])
                nc.vector.reciprocal_approx_fast(ds_[:], ds_[:])
                nc.gpsimd.partition_broadcast(
                    rb[:, 512 * s_:512 * (s_ + 1)], ds_[:])
            z = npool.tile([128, PASS], FP32, tag="z", name="z")
            nc.vector.tensor_mul(z[:], out1[:], rb[:])
            tql = npool.tile([128, PASS], FP32, tag="tq", name="tq")
            nc.scalar.activation(tql[:], z[:], ACT.Exp, scale=-1.0)
            nc.vector.tensor_scalar_add(tql[:], tql[:], 1.0)
            rsb = npool.tile([128, PASS], FP32, tag="rb", name="rb")
            nc.vector.reciprocal_approx_fast(rsb[:], tql[:])
            hT = npool.tile([128, PASS], MM_DT, tag="hT", name="hT")
            nc.vector.tensor_mul(hT[:], z[:], rsb[:])
            if h == 0 and p == 0:
                dbg_sbuf("dbg_rb", rb[:])
                dbg_sbuf("dbg_z", z[:])
                dbg_sbuf("dbg_hT", hT[:])

            # ------------- attend 2 -------------
            out2 = ps_av.tile([65, PASS], FP32, tag="av", name="av")
            masked_exp_av(k2h, hT, v2s, slice(0, 65), out2, None, p)

            # normalize attend2 (denominator rode along as row 64)
            d2 = npool.tile([1, PASS], FP32, tag="dsb", name="dsb")
            nc.vector.tensor_copy(d2[:], out2[64:65, :])
            nc.vector.tensor_scalar_add(d2[:], d2[:], sinkb[0:1, h:h + 1])
            nc.vector.reciprocal_approx_fast(d2[:], d2[:])
            rb2 = npool.tile([64, PASS], FP32, tag="rb", name="rb")
            if SKIP_GPSIMD:
                nc.vector.memset(rb2[:], 1.0)
            else:
                nc.gpsimd.partition_broadcast(rb2[:], d2[0:1, :])
            dst = o2T[h // 2][64 * (h % 2):64 * (h % 2) + 64,
                             p * PASS:(p + 1) * PASS]
            nc.vector.tensor_mul(dst, out2[0:64, :], rb2[:])
            if h == 0 and p == 0:
                dbg_psum("dbg_out2", out2[:], 65, PASS)

    # =====================================================================
    # Phase 3: partial out = o2T.T @ wout
    # =====================================================================
    for nb in range(NB):
        acc = ps_av.tile([128, PASS], FP32, tag="av", name="av")
        for s in range(2):
            for kt in range(2):
                mmr(acc[:, s * 512:(s + 1) * 512],
                    o2T[kt][:, nb * 128:(nb + 1) * 128],
                    wout_sb[kt][:, s * 512:(s + 1) * 512],
                    start=(kt == 0), stop=(kt == 1))
        osb = osb_p.tile([128, DQ], FP32, tag="osb", name="osb")
        nc.vector.tensor_copy(osb[:], acc[:])
        nc.sync.dma_start(out=out[nb * 128:(nb + 1) * 128, :], in_=osb[:])

    for p_ in reversed(_pools):
        p_.release()


_NC_CACHE = {}


def build_nc():
    key = (str(MM_DT), REPS, DEBUG, SKIP_GPSIMD, PROJ_ONLY)
    if key in _NC_CACHE:
        return _NC_CACHE[key]
    nc = bacc.Bacc("TRN2", target_bir_lowering=False, debug=False,
                   num_devices=N_CORES)
    io = {
        "xq": nc.dram_tensor("xq", [N, DQ], FP32, kind="ExternalInput").ap(),
        "xkv": nc.dram_tensor("xkv", [N, DQ], FP32, kind="ExternalInput").ap(),
        "wq": nc.dram_tensor("wq", [DQ, 256], MM_DT, kind="ExternalInput").ap(),
        "wk1": nc.dram_tensor("wk1", [DQ, 256], MM_DT, kind="ExternalInput").ap(),
        "wv1": nc.dram_tensor("wv1", [DQ, 512], MM_DT, kind="ExternalInput").ap(),
        "wk2": nc.dram_tensor("wk2", [DQ, 512], MM_DT, kind="ExternalInput").ap(),
        "wv2": nc.dram_tensor("wv2", [DQ, 256], MM_DT, kind="ExternalInput").ap(),
        "wout": nc.dram_tensor("wout", [256, DQ], MM_DT, kind="ExternalInput").ap(),
        "sink": nc.dram_tensor("sink", [1, HEADS], FP32, kind="ExternalInput").ap(),
        "out": nc.dram_tensor("out", [N, DQ], FP32, kind="ExternalOutput").ap(),
    }
    if DEBUG:
        for nm, shp in (("dbg_qT", [256, N]), ("dbg_k1T", [256, N]),
                        ("dbg_k2T", [512, N]), ("dbg_v1", [N, 512]),
                        ("dbg_v2a", [N, 260]), ("dbg_out1", [128, PASS]),
                        ("dbg_ones0", [1, 512]), ("dbg_ones1", [1, 512]),
                        ("dbg_dsb0", [1, 512]), ("dbg_dsb1", [1, 512]),
                        ("dbg_rb", [128, PASS]), ("dbg_z", [128, PASS]),
                        ("dbg_hT", [128, PASS]), ("dbg_out2", [65, PASS])):
            io[nm] = nc.dram_tensor(nm, shp, FP32, kind="ExternalOutput").ap()
    with TileContext(nc) as tc:
        if REPS == 0:
            pool0 = tc.alloc_tile_pool(name="p0", bufs=1)
            t0_ = pool0.tile([128, DQ], FP32, name="t0_")
            nc.sync.dma_start(out=t0_[:], in_=io["xq"][0:128, :])
            for nb in range(NB):
                nc.sync.dma_start(out=io["out"][nb * 128:(nb + 1) * 128, :],
                                  in_=t0_[:])
            pool0.release()
        for _ in range(REPS):
            build_kernel(nc, tc, io)
    nc.compile()
    _NC_CACHE[key] = (nc, io)
    return nc, io


def make_in_maps(inputs):
    in_maps = []
    for c in range(N_CORES):
        b, g = c // 4, c % 4
        s64 = slice(g * 256, (g + 1) * 256)
        s128 = slice(g * 512, (g + 1) * 512)
        in_maps.append({
            "xq": np.ascontiguousarray(inputs["queries_input"][b]),
            "xkv": np.ascontiguousarray(inputs["key_values_input"][b]),
            "wq": np.ascontiguousarray(inputs["Wq"][:, s64]),
            "wk1": np.ascontiguousarray(inputs["Wk1"][:, s64]),
            "wv1": np.ascontiguousarray(inputs["Wv1"][:, s128]),
            "wk2": np.ascontiguousarray(inputs["Wk2"][:, s128]),
            "wv2": np.ascontiguousarray(inputs["Wv2"][:, s64]),
            "wout": np.ascontiguousarray(inputs["Wout"][s64, :]),
            "sink": np.ascontiguousarray(
                inputs["attn_sink"][g * 4:(g + 1) * 4]).reshape(1, HEADS),
        })
    return in_maps


def kernel(**inputs):
    from concourse.bass_utils import run_bass_kernel_spmd

    inputs = {k: np.asarray(v) for k, v in inputs.items()}
    nc, _ = build_nc()
    in_maps = make_in_maps(inputs)
    res = run_bass_kernel_spmd(nc, in_maps, list(range(N_CORES)))
    out = np.zeros((2, N, DQ), dtype=np.float32)
    for c in range(N_CORES):
        out[c // 4] += res.results[c]["out"]
    return out



# revision 26
# speedup vs baseline: 650.2302x; 650.2302x over previous
"""Trainium2 Bass kernel for nn_Attention_31997506355363 (sparse_attention).

Sharding: 8 cores = 2 batches x 4 head-groups (4 heads of 16 each).
Each core computes its batch's full-sequence double-attend for its 4 heads,
plus the partial output projection (Wout rows for its heads); host sums the
4 head-group partials per batch.

Math notes (verified vs reference):
  - mask keeps j<=i OR j>i+512  (the strip i<j<=i+512 is masked out)
  - softmax has a per-head sink logit in the denominator only
  - |sim| <= ~6.4 so softmax runs without max-subtraction: p = exp(sim),
    denom = sum_j p + exp(sink)
  - attends are computed transposed: simT[j,i] tiles -> exp -> outT
    accumulated as v.T @ p per 128-j-block (contraction always on the
    partition dim, so no attention-matrix transposes are needed, and
    attend1's output hiddensT feeds attend2 directly)

Perf structure (v2):
  - all matmul operands bf16 (fp32 PE runs at 1/4 rate; tolerance is 2e-2)
  - x transposed by XBAR DMA-transpose (2-byte dtype) straight into SBUF;
    no PE transposes, no PSUM->SBUF copies for xT
  - everything SBUF-resident between phases; weights loaded once;
    phase-1-only pools (xT, projection weights, wide PSUM accs) released
    before the attends
  - projections run stationary-major (one Ldweights per (w-slice), 4
    full-width moving matmuls) to cut PE sequencer pressure
  - masking via DVE multiplies with constant 0/1 triangular tiles + DVE
    memsets; GPSIMD only does one-time constant setup
  - softmax denominators: ones-row matmuls accumulate alongside v.T @ e;
    reciprocal broadcast back to 128 partitions via a rank-1 PE matmul
"""

import sys

for _p in ("/opt/trn_rl_repo",):
    if _p not in sys.path:
        sys.path.insert(0, _p)

import numpy as np
import concourse.bass as bass
from concourse import bacc
import concourse.mybir as mybir
from concourse.tile import TileContext
from concourse.masks import make_identity

FP32 = mybir.dt.float32
MM_DT = mybir.dt.bfloat16
N_CORES = 8
N = 2048            # sequence length
DQ = 1024           # model dim
HEADS = 4           # heads per core
SCALE = 0.125       # 64 ** -0.5, folded into k1T / k2T at projection copy
NB = N // 128       # 16 key blocks
PASS = 1024         # attend i-pass width (2 passes)
ACT = mybir.ActivationFunctionType

DEBUG = False
REPS = 1
PROJ_ONLY = False   # timing experiment: stop after projections


def _runs_for(jb, p):
    """i-subblock runs (in 128-col units within a 1024-wide pass) that are
    not fully masked for key-block jb.  Sub-block t covers queries
    I = 8p + t; (I, jb) is fully masked iff 1 <= jb - I <= 3."""
    skip_lo = max(0, jb - 8 * p - 3)
    skip_hi = min(8, jb - 8 * p)
    if skip_lo >= skip_hi:
        return [(0, 8)], None
    runs = []
    if skip_lo > 0:
        runs.append((0, skip_lo))
    if skip_hi < 8:
        runs.append((skip_hi, 8))
    return runs, (skip_lo, skip_hi)


def build_kernel(nc, tc, io):
    mm = nc.tensor.matmul

    xq, xkv = io["xq"], io["xkv"]
    wq, wk1, wv1, wk2, wv2, wout, sink = (
        io["wq"], io["wk1"], io["wv1"], io["wk2"], io["wv2"], io["wout"],
        io["sink"],
    )
    out = io["out"]

    const = tc.alloc_tile_pool(name="const", bufs=1)
    stat = tc.alloc_tile_pool(name="stat", bufs=1)
    # phase-1-only pools (released before the attends)
    xt_p = tc.alloc_tile_pool(name="xt", bufs=1)
    xin = tc.alloc_tile_pool(name="xin", bufs=2)
    wpool = tc.alloc_tile_pool(name="w", bufs=1)
    ps_w = tc.alloc_tile_pool(name="ps_w", bufs=2, space="PSUM")   # 4 banks
    ps_tp = tc.alloc_tile_pool(name="ps_tp", bufs=2, space="PSUM")  # 2 banks

    ident = const.tile([128, 128], MM_DT, tag="ident", name="ident")
    make_identity(nc, ident[:])

    # ---- constants ----
    onescol = const.tile([128, 1], MM_DT, tag="onescol", name="onescol")
    nc.vector.memset(onescol[:], 1.0)
    onesrow = const.tile([1, 128], FP32, tag="onesrow", name="onesrow")
    nc.vector.memset(onesrow[:], 1.0)
    ones4 = const.tile([128, HEADS], MM_DT, tag="ones4", name="ones4")
    nc.vector.memset(ones4[:], 1.0)

    # 0/1 triangular masks (e layout is [j partitions, i cols]):
    # tri_le keeps jj <= ii (diagonal block), tri_gt keeps jj > ii (block I+4)
    tri_le = const.tile([128, 128], MM_DT, tag="tri_le", name="tri_le")
    nc.gpsimd.memset(tri_le[:], 1.0)
    nc.gpsimd.affine_select(
        out=tri_le[:], in_=tri_le[:], compare_op=mybir.AluOpType.is_ge,
        fill=0.0, base=0, pattern=[[1, 128]], channel_multiplier=-1)
    tri_gt = const.tile([128, 128], MM_DT, tag="tri_gt", name="tri_gt")
    nc.gpsimd.memset(tri_gt[:], 1.0)
    nc.gpsimd.affine_select(
        out=tri_gt[:], in_=tri_gt[:], compare_op=mybir.AluOpType.is_ge,
        fill=0.0, base=-1, pattern=[[-1, 128]], channel_multiplier=1)

    # ---- weights (DMAs ordered around the transposes; see below) ----
    def load_w(w_dram, cols, nm, eng):
        wt = [wpool.tile([128, cols], MM_DT, tag=f"{nm}{kt}", name=f"{nm}{kt}")
              for kt in range(8)]
        for kt in range(8):
            e = eng if not isinstance(eng, tuple) else eng[kt % 2]
            e.dma_start(out=wt[kt][:], in_=w_dram[kt * 128:(kt + 1) * 128, :])
        return wt

    wq_sb = load_w(wq, 256, "wq", (nc.sync, nc.scalar))

    # ---- persistent SBUF intermediates ----
    qT_sb = [stat.tile([128, N], MM_DT, tag=f"qT{t}", name=f"qT{t}") for t in range(2)]
    k1T_sb = [stat.tile([128, N], MM_DT, tag=f"k1T{t}", name=f"k1T{t}") for t in range(2)]
    k2T_sb = [stat.tile([128, N], MM_DT, tag=f"k2T{t}", name=f"k2T{t}") for t in range(4)]
    v1_sb = [stat.tile([128, 512], MM_DT, tag=f"v1_{t}", name=f"v1_{t}") for t in range(NB)]
    v2a_sb = [stat.tile([128, 65 * HEADS], MM_DT, tag=f"v2a{t}", name=f"v2a{t}")
              for t in range(NB)]
    o2T = [stat.tile([128, N], MM_DT, tag=f"o2T{t}", name=f"o2T{t}") for t in range(2)]

    # =====================================================================
    # Phase 1: DMA-transpose x into SBUF, then stationary-major projections.
    # =====================================================================
    xqT = [xt_p.tile([128, N], MM_DT, tag=f"xqT{kt}", name=f"xqT{kt}")
           for kt in range(8)]
    xkvT = [xt_p.tile([128, N], MM_DT, tag=f"xkvT{kt}", name=f"xkvT{kt}")
            for kt in range(8)]

    def transpose_chunk(x_dram, xT, c, qi):
        """PE-transpose rows [c*512, (c+1)*512) of x into xT[kt][:, c-cols].
        (The XBAR DMA-transpose path raced with compute consumers on HW —
        its completion semaphore does not reliably gate reads.)"""
        nat = []
        for nbl in range(4):
            r0 = c * 512 + nbl * 128
            t = xin.tile([128, DQ], MM_DT, tag=f"x{qi}{nbl}", name=f"x{qi}{nbl}")
            eng = nc.sync if (nbl % 2 == 0) else nc.scalar
            eng.dma_start(out=t[:], in_=x_dram[r0:r0 + 128, :])
            nat.append(t)
        for kt in range(8):
            ps = ps_tp.tile([128, 512], MM_DT, tag="tp", name="tp")
            for nbl in range(4):
                nc.tensor.transpose(
                    ps[:, nbl * 128:(nbl + 1) * 128],
                    nat[nbl][:, kt * 128:(kt + 1) * 128], ident[:])
            if kt % 2 == 0:
                nc.vector.tensor_copy(xT[kt][:, c * 512:(c + 1) * 512], ps[:])
            else:
                nc.scalar.copy(xT[kt][:, c * 512:(c + 1) * 512], ps[:])

    # remaining weights, by first use
    wk1_sb = load_w(wk1, 256, "wk1", nc.sync)
    wk2_sb = load_w(wk2, 512, "wk2", nc.scalar)
    wv1_sb = load_w(wv1, 512, "wv1", nc.sync)
    wv2_sb = load_w(wv2, 256, "wv2", nc.scalar)
    wout_sb = [stat.tile([128, DQ], MM_DT, tag=f"wo{t}", name=f"wo{t}")
               for t in range(2)]
    for t in range(2):
        nc.scalar.dma_start(out=wout_sb[t][:], in_=wout[t * 128:(t + 1) * 128, :])
    sink_sb = const.tile([1, HEADS], FP32, tag="sink", name="sink")
    nc.scalar.dma_start(out=sink_sb[:], in_=sink[:])
    esink = const.tile([1, HEADS], FP32, tag="esink", name="esink")
    nc.scalar.activation(esink[:], sink_sb[:], ACT.Exp)

    # q/k1/k2 groups: stationary-major (one Ldweights per (w-slice, kt, half),
    # two 512-wide moving matmuls); v1+v2 fused on a shared stationary.
    groups = (
        [(qT_sb[m], wq_sb, m, xqT, None) for m in range(2)]
        + [(k1T_sb[m], wk1_sb, m, xkvT, SCALE) for m in range(2)]
        + [(k2T_sb[m], wk2_sb, m, xkvT, SCALE) for m in range(4)]
    )

    def proj_groups(hf):
        cols = slice(hf * 1024, (hf + 1) * 1024)
        for gi, (dst, wsb, m, xT, scale) in enumerate(groups):
            acc = ps_w.tile([128, PASS], FP32, tag="pw", name="pw")
            for kt in range(8):
                for cb in range(2):
                    c0 = hf * 1024 + cb * 512
                    mm(acc[:, cb * 512:(cb + 1) * 512],
                       wsb[kt][:, m * 128:(m + 1) * 128],
                       xT[kt][:, c0:c0 + 512],
                       start=(kt == 0), stop=(kt == 7))
            if scale is None:
                if gi % 2 == 0:
                    nc.vector.tensor_copy(dst[:, cols], acc[:])
                else:
                    nc.scalar.copy(dst[:, cols], acc[:])
            else:
                if gi % 2 == 0:
                    nc.vector.tensor_scalar_mul(dst[:, cols], acc[:], scale)
                else:
                    nc.scalar.mul(dst[:, cols], acc[:], scale)

    def proj_v(hf):
        for nb in range(8 * hf, 8 * hf + 8):
            acc = ps_w.tile([128, PASS], FP32, tag="pw", name="pw")
            for kt in range(8):
                mm(acc[:, 0:512], xkvT[kt][:, nb * 128:(nb + 1) * 128], wv1_sb[kt][:],
                   start=(kt == 0), stop=(kt == 7))
                mm(acc[:, 512:768], xkvT[kt][:, nb * 128:(nb + 1) * 128], wv2_sb[kt][:],
                   start=(kt == 0), stop=(kt == 7))
            if nb % 2 == 0:
                nc.vector.tensor_copy(v1_sb[nb][:], acc[:, 0:512])
            else:
                nc.scalar.copy(v1_sb[nb][:], acc[:, 0:512])
            # pack v2 [h*64 cols] into 65-col groups with a ones column
            sv = v2a_sb[nb][:].rearrange("p (h c) -> p h c", h=HEADS)
            nc.vector.tensor_copy(
                sv[:, :, 0:64],
                acc[:, 512:768].rearrange("p (h c) -> p h c", h=HEADS))
            nc.vector.tensor_copy(
                sv[:, :, 64:65],
                ones4[:].rearrange("p (h c) -> p h c", h=HEADS))

    for hf in range(2):
        transpose_chunk(xq, xqT, 2 * hf, "q")
        transpose_chunk(xkv, xkvT, 2 * hf, "k")
        transpose_chunk(xq, xqT, 2 * hf + 1, "q")
        transpose_chunk(xkv, xkvT, 2 * hf + 1, "k")
        proj_groups(hf)
        proj_v(hf)

    ps_tp.release()
    ps_w.release()
    wpool.release()
    xin.release()
    xt_p.release()

    # attend-phase pools (allocated after the phase-1 pools are released)
    e1p = tc.alloc_tile_pool(name="e1", bufs=1)    # 16 resident e tiles
    epool = tc.alloc_tile_pool(name="e", bufs=3)
    npool = tc.alloc_tile_pool(name="nrm", bufs=2)
    osb_p = tc.alloc_tile_pool(name="osb", bufs=2)
    ps_a = tc.alloc_tile_pool(name="ps_a", bufs=2, space="PSUM")   # 4 banks
    ps_b = tc.alloc_tile_pool(name="ps_b", bufs=1, space="PSUM")   # 2 banks
    ps_on = tc.alloc_tile_pool(name="ps_on", bufs=1, space="PSUM")  # 1 bank
    ps_bc = tc.alloc_tile_pool(name="ps_bc", bufs=1, space="PSUM")  # 1 bank
    _pools2 = [e1p, epool, npool, osb_p, ps_a, ps_b, ps_on, ps_bc]

    if PROJ_ONLY:
        for nb in range(NB):
            osb = osb_p.tile([128, DQ], FP32, tag="osb", name="osb")
            nc.vector.tensor_copy(osb[:, 0:512], v1_sb[nb][:])
            nc.vector.tensor_copy(osb[:, 512:1024], v1_sb[nb][:])
            nc.sync.dma_start(out=out[nb * 128:(nb + 1) * 128, :], in_=osb[:])
        for p_ in reversed(_pools2):
            p_.release()
        for p_ in (stat, const):
            p_.release()
        return

    # =====================================================================
    # Phase 2: attends (everything SBUF-resident)
    # =====================================================================
    def masked_exp_av(k_h, rhs_h, v_ap, out_ps, ones_ps, p):
        """One attend pass: for each key block jb, sim -> exp -> mask ->
        accumulate v.T @ e (and the ones row for attend1 denominators).

        Software-pipelined one jb deep: the PE emission order is
        sim(0), sim(1), av(0), sim(2), av(1), ... so the in-order PE queue
        never stalls on exp/mask of the block it is about to accumulate."""
        def do_sim(jb):
            simp = ps_a.tile([128, PASS], FP32, tag="sim", name="sim")
            for col in (0, 512):
                mm(simp[:, col:col + 512],
                   k_h[:, jb * 128:(jb + 1) * 128],
                   rhs_h[:, col:col + 512],
                   start=True, stop=True)
            return simp

        def do_e(jb, simp):
            _, skip = _runs_for(jb, p)
            e = epool.tile([128, PASS], MM_DT, tag="e", name="e")
            nc.scalar.activation(e[:], simp[:], ACT.Exp)
            if skip is not None:
                nc.vector.memset(e[:, skip[0] * 128:skip[1] * 128], 0.0)
            td = jb - 8 * p
            if 0 <= td < 8:   # diagonal block: keep jj <= ii
                blk = e[:, td * 128:(td + 1) * 128]
                nc.vector.tensor_mul(blk, blk, tri_le[:])
            ta = jb - 4 - 8 * p
            if 0 <= ta < 8:   # jb == I+4 block: keep jj > ii
                blk = e[:, ta * 128:(ta + 1) * 128]
                nc.vector.tensor_mul(blk, blk, tri_gt[:])
            return e

        def do_av(jb, e):
            for s in range(2):
                mm(out_ps[:, s * 512:(s + 1) * 512],
                   v_ap(jb),
                   e[:, s * 512:(s + 1) * 512],
                   start=(jb == 0), stop=(jb == NB - 1))
            if ones_ps is not None:
                for s in range(2):
                    mm(ones_ps[32 * s:32 * s + 1, :], onescol[:],
                       e[:, s * 512:(s + 1) * 512],
                       start=(jb == 0), stop=(jb == NB - 1))

        prev = None
        for jb in range(NB):
            simp = do_sim(jb)
            if prev is not None:
                do_av(jb - 1, prev)
            prev = do_e(jb, simp)
        do_av(NB - 1, prev)

    def sim_exp_1(h, p):
        """Attend1 S-stage: sims -> exp -> mask into 16 resident e tiles.
        Emitted one pass ahead so the PE has independent work during the
        previous pass's normalization chain."""
        hh = 64 * (h % 2)
        k1h = k1T_sb[h // 2][hh:hh + 64, :]
        qh = qT_sb[h // 2][hh:hh + 64, p * PASS:(p + 1) * PASS]
        es = []
        for jb in range(NB):
            simp = ps_a.tile([128, PASS], FP32, tag="sim", name="sim")
            for col in (0, 512):
                mm(simp[:, col:col + 512],
                   k1h[:, jb * 128:(jb + 1) * 128],
                   qh[:, col:col + 512],
                   start=True, stop=True)
            e = e1p.tile([128, PASS], MM_DT, tag=f"e1_{jb}", name=f"e1_{jb}")
            nc.scalar.activation(e[:], simp[:], ACT.Exp)
            _, skip = _runs_for(jb, p)
            if skip is not None:
                nc.vector.memset(e[:, skip[0] * 128:skip[1] * 128], 0.0)
            td = jb - 8 * p
            if 0 <= td < 8:
                blk = e[:, td * 128:(td + 1) * 128]
                nc.vector.tensor_mul(blk, blk, tri_le[:])
            ta = jb - 4 - 8 * p
            if 0 <= ta < 8:
                blk = e[:, ta * 128:(ta + 1) * 128]
                nc.vector.tensor_mul(blk, blk, tri_gt[:])
            es.append(e)
        return es

    def wout_half(p):
        """Phase 3 for the column half finished by pass group p."""
        for nb in range(8 * p, 8 * p + 8):
            pool, tag = (ps_b, "av") if nb % 2 == 0 else (ps_a, "sim")
            acc = pool.tile([128, PASS], FP32, tag=tag, name=tag)
            for s in range(2):
                for kt in range(2):
                    mm(acc[:, s * 512:(s + 1) * 512],
                       o2T[kt][:, nb * 128:(nb + 1) * 128],
                       wout_sb[kt][:, s * 512:(s + 1) * 512],
                       start=(kt == 0), stop=(kt == 1))
            osb = osb_p.tile([128, DQ], FP32, tag="osb", name="osb")
            if nb % 2 == 0:
                nc.vector.tensor_copy(osb[:], acc[:])
            else:
                nc.scalar.copy(osb[:], acc[:])
            nc.sync.dma_start(out=out[nb * 128:(nb + 1) * 128, :], in_=osb[:])

    passes = [(h, p) for p in range(2) for h in range(HEADS)]
    e1s = sim_exp_1(*passes[0])
    for idx, (h, p) in enumerate(passes):
        hh = 64 * (h % 2)

        # ------------- attend 1 V-stage: av + denominator matmuls ---------
        out1 = ps_b.tile([128, PASS], FP32, tag="av", name="av")
        ones = ps_on.tile([33, 512], FP32, tag="ones", name="ones")
        for jb in range(NB):
            for s in range(2):
                mm(out1[:, s * 512:(s + 1) * 512],
                   v1_sb[jb][:, 128 * h:128 * h + 128],
                   e1s[jb][:, s * 512:(s + 1) * 512],
                   start=(jb == 0), stop=(jb == NB - 1))
            for s in range(2):
                mm(ones[32 * s:32 * s + 1, :], onescol[:],
                   e1s[jb][:, s * 512:(s + 1) * 512],
                   start=(jb == 0), stop=(jb == NB - 1))

        # normalize (z = out1 / denom) + silu -> hT
        zf = npool.tile([128, PASS], FP32, tag="z", name="z")
        rbs = npool.tile([128, PASS], FP32, tag="rb", name="rb")
        for s_ in range(2):
            ds_ = npool.tile([1, PASS], FP32, tag="ds", name="ds")
            nc.vector.tensor_copy(ds_[0:1, 0:512], ones[32 * s_:32 * s_ + 1, :])
            nc.vector.tensor_scalar_add(ds_[0:1, 0:512], ds_[0:1, 0:512],
                                        esink[0:1, h:h + 1])
            nc.vector.reciprocal_approx_fast(ds_[0:1, 0:512], ds_[0:1, 0:512])
            rbp = ps_bc.tile([128, 512], FP32, tag="bc", name="bc")
            mm(rbp[:], onesrow[:], ds_[0:1, 0:512], start=True, stop=True)
            nc.scalar.copy(rbs[:, s_ * 512:(s_ + 1) * 512], rbp[:])
        nc.vector.tensor_mul(zf[:], out1[:], rbs[:])
        # silu(z) = z * sigmoid(z) = z / (1 + exp(-z)); stays in the
        # Exp activation table (Silu lives in a different table)
        tql = npool.tile([128, PASS], FP32, tag="tq", name="tq")
        nc.scalar.activation(tql[:], zf[:], ACT.Exp, scale=-1.0)
        nc.vector.tensor_scalar_add(tql[:], tql[:], 1.0)
        nc.vector.reciprocal_approx_fast(tql[:], tql[:])
        hT = npool.tile([128, PASS], MM_DT, tag="hT", name="hT")
        nc.vector.tensor_mul(hT[:], zf[:], tql[:])

        # next pass's S-stage: fills the PE while the chain above runs
        if idx + 1 < len(passes):
            e1s = sim_exp_1(*passes[idx + 1])

        # ------------- attend 2 (fused jb-pipelined) -------------
        k2h = k2T_sb[h][:]
        out2 = ps_b.tile([65, PASS], FP32, tag="av", name="av")
        masked_exp_av(
            k2h, hT[:], lambda jb: v2a_sb[jb][:, 65 * h:65 * h + 65],
            out2[:], None, p)

        # normalize attend2 (denominator rode along as row 64)
        d2 = npool.tile([1, PASS], FP32, tag="ds", name="ds")
        nc.vector.tensor_copy(d2[:], out2[64:65, :])
        nc.vector.tensor_scalar_add(d2[:], d2[:], esink[0:1, h:h + 1])
        nc.vector.reciprocal_approx_fast(d2[:], d2[:])
        rbs2 = npool.tile([64, PASS], FP32, tag="rb2", name="rb2")
        for s_ in range(2):
            rbp = ps_bc.tile([128, 512], FP32, tag="bc", name="bc")
            mm(rbp[0:64, :], onesrow[0:1, 0:64],
               d2[0:1, s_ * 512:(s_ + 1) * 512], start=True, stop=True)
            nc.scalar.copy(rbs2[:, s_ * 512:(s_ + 1) * 512], rbp[0:64, :])
        dst = o2T[h // 2][hh:hh + 64, p * PASS:(p + 1) * PASS]
        nc.vector.tensor_mul(dst, out2[0:64, :], rbs2[:])

        if DEBUG and h == 0 and p == 0:
            nc.sync.dma_start(out=io["dbg_hT"].bitcast(MM_DT), in_=hT[:])
            dzf = npool.tile([128, PASS], FP32, tag="dzf", name="dzf")
            nc.vector.tensor_copy(dzf[:], zf[:])
            nc.sync.dma_start(out=io["dbg_zf"], in_=dzf[:])
            do2 = npool.tile([65, PASS], FP32, tag="do2", name="do2")
            nc.vector.tensor_copy(do2[:], out2[:])
            nc.sync.dma_start(out=io["dbg_out2"], in_=do2[:])

        # interleave the output projection for the completed column half
        if idx == len(passes) - 1 or (idx + 1 < len(passes)
                                      and passes[idx + 1][1] != p):
            wout_half(p)

    if DEBUG:
        for t in range(2):
            nc.sync.dma_start(out=io["dbg_qT"][t * 128:(t + 1) * 128, :].bitcast(MM_DT),
                              in_=qT_sb[t][:])
            nc.sync.dma_start(out=io["dbg_k1T"][t * 128:(t + 1) * 128, :].bitcast(MM_DT),
                              in_=k1T_sb[t][:])
            nc.sync.dma_start(out=io["dbg_o2T"][t * 128:(t + 1) * 128, :].bitcast(MM_DT),
                              in_=o2T[t][:])
        for t in range(4):
            nc.sync.dma_start(out=io["dbg_v1"][t * 128:(t + 1) * 128, :].bitcast(MM_DT),
                              in_=v1_sb[t][:])

    for p_ in reversed(_pools2):
        p_.release()
    for p_ in (stat, const):
        p_.release()


_NC_CACHE = {}


def build_nc():
    key = (str(MM_DT), REPS, DEBUG, PROJ_ONLY)
    if key in _NC_CACHE:
        return _NC_CACHE[key]
    nc = bacc.Bacc("TRN2", target_bir_lowering=False, debug=False,
                   num_devices=N_CORES)
    io = {
        "xq": nc.dram_tensor("xq", [N, DQ], MM_DT, kind="ExternalInput").ap(),
        "xkv": nc.dram_tensor("xkv", [N, DQ], MM_DT, kind="ExternalInput").ap(),
        "wq": nc.dram_tensor("wq", [DQ, 256], MM_DT, kind="ExternalInput").ap(),
        "wk1": nc.dram_tensor("wk1", [DQ, 256], MM_DT, kind="ExternalInput").ap(),
        "wv1": nc.dram_tensor("wv1", [DQ, 512], MM_DT, kind="ExternalInput").ap(),
        "wk2": nc.dram_tensor("wk2", [DQ, 512], MM_DT, kind="ExternalInput").ap(),
        "wv2": nc.dram_tensor("wv2", [DQ, 256], MM_DT, kind="ExternalInput").ap(),
        "wout": nc.dram_tensor("wout", [256, DQ], MM_DT, kind="ExternalInput").ap(),
        "sink": nc.dram_tensor("sink", [1, HEADS], FP32, kind="ExternalInput").ap(),
        "out": nc.dram_tensor("out", [N, DQ], FP32, kind="ExternalOutput").ap(),
    }
    if DEBUG:
        for nm, shp, dt in (("dbg_qT", [256, N], FP32), ("dbg_k1T", [256, N], FP32),
                            ("dbg_o2T", [256, N], FP32), ("dbg_v1", [512, 512], FP32),
                            ("dbg_hT", [128, PASS], FP32), ("dbg_zf", [128, PASS], FP32),
                            ("dbg_out2", [65, PASS], FP32)):
            shp2 = list(shp)
            if dt is FP32 and nm in ("dbg_qT", "dbg_k1T", "dbg_o2T", "dbg_v1", "dbg_hT"):
                shp2[-1] = shp[-1] // 2   # bf16 payload bitcast into fp32 words
            io[nm] = nc.dram_tensor(nm, shp2, FP32, kind="ExternalOutput").ap()
    if REPS == 0:
        # extra input so the I/O-only program's jax trace-cache key differs
        # from the real kernel's (the cache ignores the BIR payload)
        io["dummy0"] = nc.dram_tensor("dummy0", [1, 8], FP32,
                                      kind="ExternalInput").ap()
    with TileContext(nc) as tc:
        if REPS == 0:
            pool0 = tc.alloc_tile_pool(name="p0", bufs=1)
            t0_ = pool0.tile([128, DQ], MM_DT, name="t0_")
            nc.sync.dma_start(out=t0_[:], in_=io["xq"][0:128, :])
            o0_ = pool0.tile([128, DQ], FP32, name="o0_")
            nc.vector.tensor_copy(o0_[:], t0_[:])
            for nb in range(NB):
                nc.sync.dma_start(out=io["out"][nb * 128:(nb + 1) * 128, :],
                                  in_=o0_[:])
            pool0.release()
        for _ in range(REPS):
            build_kernel(nc, tc, io)
    nc.compile()
    _NC_CACHE[key] = (nc, io)
    return nc, io


_BF16 = None


def _bf16():
    global _BF16
    if _BF16 is None:
        import ml_dtypes
        _BF16 = np.dtype(ml_dtypes.bfloat16)
    return _BF16


def make_in_maps(inputs):
    bf = _bf16()
    xq_b = [np.ascontiguousarray(inputs["queries_input"][b]).astype(bf)
            for b in range(2)]
    xkv_b = [np.ascontiguousarray(inputs["key_values_input"][b]).astype(bf)
             for b in range(2)]
    in_maps = []
    for c in range(N_CORES):
        b, g = c // 4, c % 4
        s64 = slice(g * 256, (g + 1) * 256)
        s128 = slice(g * 512, (g + 1) * 512)
        in_maps.append({
            "xq": xq_b[b],
            "xkv": xkv_b[b],
            "wq": np.ascontiguousarray(inputs["Wq"][:, s64]).astype(bf),
            "wk1": np.ascontiguousarray(inputs["Wk1"][:, s64]).astype(bf),
            "wv1": np.ascontiguousarray(inputs["Wv1"][:, s128]).astype(bf),
            "wk2": np.ascontiguousarray(inputs["Wk2"][:, s128]).astype(bf),
            "wv2": np.ascontiguousarray(inputs["Wv2"][:, s64]).astype(bf),
            "wout": np.ascontiguousarray(inputs["Wout"][s64, :]).astype(bf),
            "sink": np.ascontiguousarray(
                inputs["attn_sink"][g * 4:(g + 1) * 4]).reshape(1, HEADS)
                .astype(np.float32),
        })
    return in_maps


def kernel(**inputs):
    from concourse.bass_utils import run_bass_kernel_spmd

    inputs = {k: np.asarray(v) for k, v in inputs.items()}
    nc, _ = build_nc()
    in_maps = make_in_maps(inputs)
    res = run_bass_kernel_spmd(nc, in_maps, list(range(N_CORES)))
    out = np.zeros((2, N, DQ), dtype=np.float32)
    for c in range(N_CORES):
        out[c // 4] += res.results[c]["out"]
    return out


# revision 34
# speedup vs baseline: 756.1651x; 1.1629x over previous
"""Trainium2 Bass kernel for nn_Attention_31997506355363 (sparse_attention).

Sharding: 8 cores = 2 batches x 4 head-groups (4 heads of 16 each).
Each core computes its batch's full-sequence double-attend for its 4 heads,
plus the partial output projection (Wout rows for its heads); host sums the
4 head-group partials per batch.

Math notes (verified vs reference):
  - mask keeps j<=i OR j>i+512  (the strip i<j<=i+512 is masked out)
  - softmax has a per-head sink logit in the denominator only
  - |sim| <= ~6.4 so softmax runs without max-subtraction: p = exp(sim),
    denom = sum_j p + exp(sink)
  - attends are computed transposed: simT[j,i] tiles -> exp -> outT
    accumulated as v.T @ p per 128-j-block (contraction always on the
    partition dim, so no attention-matrix transposes are needed, and
    attend1's output hiddensT feeds attend2 directly)

Perf structure (v2):
  - all matmul operands bf16 (fp32 PE runs at 1/4 rate; tolerance is 2e-2)
  - x transposed by XBAR DMA-transpose (2-byte dtype) straight into SBUF;
    no PE transposes, no PSUM->SBUF copies for xT
  - everything SBUF-resident between phases; weights loaded once;
    phase-1-only pools (xT, projection weights, wide PSUM accs) released
    before the attends
  - projections run stationary-major (one Ldweights per (w-slice), 4
    full-width moving matmuls) to cut PE sequencer pressure
  - masking via DVE multiplies with constant 0/1 triangular tiles + DVE
    memsets; GPSIMD only does one-time constant setup
  - softmax denominators: ones-row matmuls accumulate alongside v.T @ e;
    reciprocal broadcast back to 128 partitions via a rank-1 PE matmul
"""

import sys

for _p in ("/opt/trn_rl_repo",):
    if _p not in sys.path:
        sys.path.insert(0, _p)

import numpy as np
import concourse.bass as bass
from concourse import bacc
import concourse.mybir as mybir
from concourse.tile import TileContext
from concourse.masks import make_identity

FP32 = mybir.dt.float32
MM_DT = mybir.dt.bfloat16
N_CORES = 8
N = 2048            # sequence length
DQ = 1024           # model dim
HEADS = 4           # heads per core
SCALE = 0.125       # 64 ** -0.5, folded into k1T / k2T at projection copy
NB = N // 128       # 16 key blocks
PASS = 1024         # attend i-pass width (2 passes)
ACT = mybir.ActivationFunctionType

DEBUG = False
REPS = 1
PROJ_ONLY = False   # timing experiment: stop after projections


def _runs_for(jb, p):
    """i-subblock runs (in 128-col units within a 1024-wide pass) that are
    not fully masked for key-block jb.  Sub-block t covers queries
    I = 8p + t; (I, jb) is fully masked iff 1 <= jb - I <= 3."""
    skip_lo = max(0, jb - 8 * p - 3)
    skip_hi = min(8, jb - 8 * p)
    if skip_lo >= skip_hi:
        return [(0, 8)], None
    runs = []
    if skip_lo > 0:
        runs.append((0, skip_lo))
    if skip_hi < 8:
        runs.append((skip_hi, 8))
    return runs, (skip_lo, skip_hi)


def build_kernel(nc, tc, io):
    mm = nc.tensor.matmul

    xq, xkv = io["xq"], io["xkv"]
    wq, wk1, wv1, wk2, wv2, wout, sink = (
        io["wq"], io["wk1"], io["wv1"], io["wk2"], io["wv2"], io["wout"],
        io["sink"],
    )
    out = io["out"]

    const = tc.alloc_tile_pool(name="const", bufs=1)
    stat = tc.alloc_tile_pool(name="stat", bufs=1)
    # phase-1-only pools (released before the attends)
    xt_p = tc.alloc_tile_pool(name="xt", bufs=1)
    xin = tc.alloc_tile_pool(name="xin", bufs=1)
    wpool = tc.alloc_tile_pool(name="w", bufs=1)
    ps_w = tc.alloc_tile_pool(name="ps_w", bufs=2, space="PSUM")   # 4 banks
    ps_tp = tc.alloc_tile_pool(name="ps_tp", bufs=2, space="PSUM")  # 2 banks

    ident = const.tile([128, 128], MM_DT, tag="ident", name="ident")
    make_identity(nc, ident[:])

    # ---- constants ----
    onescol = const.tile([128, 1], MM_DT, tag="onescol", name="onescol")
    nc.vector.memset(onescol[:], 1.0)
    onesrow = const.tile([1, 128], FP32, tag="onesrow", name="onesrow")
    nc.vector.memset(onesrow[:], 1.0)
    ones4 = const.tile([128, HEADS], MM_DT, tag="ones4", name="ones4")
    nc.vector.memset(ones4[:], 1.0)

    # 0/1 triangular masks (e layout is [j partitions, i cols]):
    # tri_le keeps jj <= ii (diagonal block), tri_gt keeps jj > ii (block I+4)
    tri_le = const.tile([128, 128], MM_DT, tag="tri_le", name="tri_le")
    nc.gpsimd.memset(tri_le[:], 1.0)
    nc.gpsimd.affine_select(
        out=tri_le[:], in_=tri_le[:], compare_op=mybir.AluOpType.is_ge,
        fill=0.0, base=0, pattern=[[1, 128]], channel_multiplier=-1)
    tri_gt = const.tile([128, 128], MM_DT, tag="tri_gt", name="tri_gt")
    nc.gpsimd.memset(tri_gt[:], 1.0)
    nc.gpsimd.affine_select(
        out=tri_gt[:], in_=tri_gt[:], compare_op=mybir.AluOpType.is_ge,
        fill=0.0, base=-1, pattern=[[-1, 128]], channel_multiplier=1)

    # ---- weights (DMAs ordered around the transposes; see below) ----
    def load_w(w_dram, cols, nm, eng):
        wt = [wpool.tile([128, cols], MM_DT, tag=f"{nm}{kt}", name=f"{nm}{kt}")
              for kt in range(8)]
        for kt in range(8):
            e = eng if not isinstance(eng, tuple) else eng[kt % 2]
            e.dma_start(out=wt[kt][:], in_=w_dram[kt * 128:(kt + 1) * 128, :])
        return wt

    wq_sb = load_w(wq, 256, "wq", (nc.sync, nc.scalar))

    # ---- persistent SBUF intermediates ----
    qT_sb = [stat.tile([128, N], MM_DT, tag=f"qT{t}", name=f"qT{t}") for t in range(2)]
    k1T_sb = [stat.tile([128, N], MM_DT, tag=f"k1T{t}", name=f"k1T{t}") for t in range(2)]
    k2T_sb = [stat.tile([128, N], MM_DT, tag=f"k2T{t}", name=f"k2T{t}") for t in range(4)]
    v1_sb = [stat.tile([128, 512], MM_DT, tag=f"v1_{t}", name=f"v1_{t}") for t in range(NB)]
    v2a_sb = [stat.tile([128, 65 * HEADS], MM_DT, tag=f"v2a{t}", name=f"v2a{t}")
              for t in range(NB)]
    o2T = [stat.tile([128, N], MM_DT, tag=f"o2T{t}", name=f"o2T{t}") for t in range(2)]

    # =====================================================================
    # Phase 1: DMA-transpose x into SBUF, then stationary-major projections.
    # =====================================================================
    xqT = [xt_p.tile([128, N], MM_DT, tag=f"xqT{kt}", name=f"xqT{kt}")
           for kt in range(8)]
    xkvT = [xt_p.tile([128, N], MM_DT, tag=f"xkvT{kt}", name=f"xkvT{kt}")
            for kt in range(8)]

    def load_chunk(x_dram, c, qi):
        nat = []
        for nbl in range(4):
            r0 = c * 512 + nbl * 128
            t = xin.tile([128, DQ], MM_DT, tag=f"x{qi}{nbl}", name=f"x{qi}{nbl}")
            eng = nc.sync if (nbl % 2 == 0) else nc.scalar
            eng.dma_start(out=t[:], in_=x_dram[r0:r0 + 128, :])
            nat.append(t)
        return nat

    def transpose_nat(nat, xT, c):
        """PE-transpose a loaded 512-row chunk into xT[kt][:, c-cols].
        (The XBAR DMA-transpose path raced with compute consumers on HW —
        its completion semaphore does not reliably gate reads.)"""
        for kt in range(8):
            ps = ps_tp.tile([128, 512], MM_DT, tag="tp", name="tp")
            for nbl in range(4):
                nc.tensor.transpose(
                    ps[:, nbl * 128:(nbl + 1) * 128],
                    nat[nbl][:, kt * 128:(kt + 1) * 128], ident[:])
            if kt % 2 == 0:
                nc.vector.tensor_copy(xT[kt][:, c * 512:(c + 1) * 512], ps[:])
            else:
                nc.scalar.copy(xT[kt][:, c * 512:(c + 1) * 512], ps[:])

    def load_rest_of_weights():
        # emitted after the first chunk's x loads so the PE isn't starved
        # at startup waiting for transposable data behind 40 weight DMAs
        w = {}
        w["k1"] = load_w(wk1, 256, "wk1", nc.sync)
        w["k2"] = load_w(wk2, 512, "wk2", nc.scalar)
        w["v1"] = load_w(wv1, 512, "wv1", nc.sync)
        w["v2"] = load_w(wv2, 256, "wv2", nc.scalar)
        w["out"] = [stat.tile([128, DQ], MM_DT, tag=f"wo{t}", name=f"wo{t}")
                    for t in range(2)]
        for t in range(2):
            nc.scalar.dma_start(out=w["out"][t][:],
                                in_=wout[t * 128:(t + 1) * 128, :])
        sink_sb = const.tile([1, HEADS], FP32, tag="sink", name="sink")
        nc.scalar.dma_start(out=sink_sb[:], in_=sink[:])
        esink = const.tile([1, HEADS], FP32, tag="esink", name="esink")
        nc.scalar.activation(esink[:], sink_sb[:], ACT.Exp)
        return w, esink

    # q/k1/k2 groups: stationary-major (one Ldweights per (w-slice, kt, half),
    # two 512-wide moving matmuls); v1+v2 fused on a shared stationary.
    def proj_groups(hf):
        groups = (
            [(qT_sb[m], wq_sb, m, xqT, None) for m in range(2)]
            + [(k1T_sb[m], wk1_sb, m, xkvT, SCALE) for m in range(2)]
            + [(k2T_sb[m], wk2_sb, m, xkvT, SCALE) for m in range(4)]
        )
        cols = slice(hf * 1024, (hf + 1) * 1024)
        for gi, (dst, wsb, m, xT, scale) in enumerate(groups):
            acc = ps_w.tile([128, PASS], FP32, tag="pw", name="pw")
            for kt in range(8):
                for cb in range(2):
                    c0 = hf * 1024 + cb * 512
                    mm(acc[:, cb * 512:(cb + 1) * 512],
                       wsb[kt][:, m * 128:(m + 1) * 128],
                       xT[kt][:, c0:c0 + 512],
                       start=(kt == 0), stop=(kt == 7))
            if scale is None:
                if gi % 2 == 0:
                    nc.vector.tensor_copy(dst[:, cols], acc[:])
                else:
                    nc.scalar.copy(dst[:, cols], acc[:])
            else:
                if gi % 2 == 0:
                    nc.vector.tensor_scalar_mul(dst[:, cols], acc[:], scale)
                else:
                    nc.scalar.mul(dst[:, cols], acc[:], scale)

    def proj_v(hf):
        for nb in range(8 * hf, 8 * hf + 8):
            acc = ps_w.tile([128, PASS], FP32, tag="pw", name="pw")
            for kt in range(8):
                mm(acc[:, 0:512], xkvT[kt][:, nb * 128:(nb + 1) * 128], wv1_sb[kt][:],
                   start=(kt == 0), stop=(kt == 7))
                mm(acc[:, 512:768], xkvT[kt][:, nb * 128:(nb + 1) * 128], wv2_sb[kt][:],
                   start=(kt == 0), stop=(kt == 7))
            if nb % 2 == 0:
                nc.vector.tensor_copy(v1_sb[nb][:], acc[:, 0:512])
            else:
                nc.scalar.copy(v1_sb[nb][:], acc[:, 0:512])
            # pack v2 [h*64 cols] into 65-col groups with a ones column
            sv = v2a_sb[nb][:].rearrange("p (h c) -> p h c", h=HEADS)
            nc.vector.tensor_copy(
                sv[:, :, 0:64],
                acc[:, 512:768].rearrange("p (h c) -> p h c", h=HEADS))
            nc.vector.tensor_copy(
                sv[:, :, 64:65],
                ones4[:].rearrange("p (h c) -> p h c", h=HEADS))

    natq0 = load_chunk(xq, 0, "q")
    natk0 = load_chunk(xkv, 0, "k")
    natq1 = load_chunk(xq, 1, "q2")
    natk1 = load_chunk(xkv, 1, "k2")
    transpose_nat(natq0, xqT, 0)
    transpose_nat(natk0, xkvT, 0)
    _w, esink = load_rest_of_weights()
    wk1_sb, wk2_sb, wv1_sb, wv2_sb, wout_sb = (
        _w["k1"], _w["k2"], _w["v1"], _w["v2"], _w["out"])
    transpose_nat(natq1, xqT, 1)
    transpose_nat(natk1, xkvT, 1)
    natq2 = load_chunk(xq, 2, "q")
    natk2 = load_chunk(xkv, 2, "k")
    natq3 = load_chunk(xq, 3, "q2")
    natk3 = load_chunk(xkv, 3, "k2")
    proj_groups(0)
    proj_v(0)
    transpose_nat(natq2, xqT, 2)
    transpose_nat(natk2, xkvT, 2)
    transpose_nat(natq3, xqT, 3)
    transpose_nat(natk3, xkvT, 3)
    proj_groups(1)
    proj_v(1)

    ps_tp.release()
    ps_w.release()
    wpool.release()
    xin.release()
    xt_p.release()

    # attend-phase pools (allocated after the phase-1 pools are released)
    e1p = tc.alloc_tile_pool(name="e1", bufs=1)    # 16 resident e tiles
    epool = tc.alloc_tile_pool(name="e", bufs=3)
    npool = tc.alloc_tile_pool(name="nrm", bufs=2)
    osb_p = tc.alloc_tile_pool(name="osb", bufs=2)
    ps_a = tc.alloc_tile_pool(name="ps_a", bufs=2, space="PSUM")   # 4 banks
    ps_b = tc.alloc_tile_pool(name="ps_b", bufs=1, space="PSUM")   # 2 banks
    ps_on = tc.alloc_tile_pool(name="ps_on", bufs=1, space="PSUM")  # 1 bank
    ps_bc = tc.alloc_tile_pool(name="ps_bc", bufs=1, space="PSUM")  # 1 bank
    _pools2 = [e1p, epool, npool, osb_p, ps_a, ps_b, ps_on, ps_bc]

    if PROJ_ONLY:
        for nb in range(NB):
            osb = osb_p.tile([128, DQ], FP32, tag="osb", name="osb")
            nc.vector.tensor_copy(osb[:, 0:512], v1_sb[nb][:])
            nc.vector.tensor_copy(osb[:, 512:1024], v1_sb[nb][:])
            nc.sync.dma_start(out=out[nb * 128:(nb + 1) * 128, :], in_=osb[:])
        for p_ in reversed(_pools2):
            p_.release()
        for p_ in (stat, const):
            p_.release()
        return

    # =====================================================================
    # Phase 2: attends (everything SBUF-resident)
    # =====================================================================
    def masked_exp_av(k_h, rhs_h, v_ap, out_ps, ones_ps, p):
        """One attend pass: for each key block jb, sim -> exp -> mask ->
        accumulate v.T @ e (and the ones row for attend1 denominators).

        Software-pipelined one jb deep: the PE emission order is
        sim(0), sim(1), av(0), sim(2), av(1), ... so the in-order PE queue
        never stalls on exp/mask of the block it is about to accumulate."""
        def do_sim(jb):
            simp = ps_a.tile([128, PASS], FP32, tag="sim", name="sim")
            for col in (0, 512):
                mm(simp[:, col:col + 512],
                   k_h[:, jb * 128:(jb + 1) * 128],
                   rhs_h[:, col:col + 512],
                   start=True, stop=True)
            return simp

        def do_e(jb, simp):
            runs, skip = _runs_for(jb, p)
            e = epool.tile([128, PASS], MM_DT, tag="e", name="e")
            for (t0, t1) in runs:
                nc.scalar.activation(e[:, t0 * 128:t1 * 128],
                                     simp[:, t0 * 128:t1 * 128], ACT.Exp)
            if skip is not None:
                nc.vector.memset(e[:, skip[0] * 128:skip[1] * 128], 0.0)
            td = jb - 8 * p
            if 0 <= td < 8:   # diagonal block: keep jj <= ii
                blk = e[:, td * 128:(td + 1) * 128]
                nc.vector.tensor_mul(blk, blk, tri_le[:])
            ta = jb - 4 - 8 * p
            if 0 <= ta < 8:   # jb == I+4 block: keep jj > ii
                blk = e[:, ta * 128:(ta + 1) * 128]
                nc.vector.tensor_mul(blk, blk, tri_gt[:])
            return e

        def do_av(jb, e):
            for s in range(2):
                mm(out_ps[:, s * 512:(s + 1) * 512],
                   v_ap(jb),
                   e[:, s * 512:(s + 1) * 512],
                   start=(jb == 0), stop=(jb == NB - 1))
            if ones_ps is not None:
                for s in range(2):
                    mm(ones_ps[32 * s:32 * s + 1, :], onescol[:],
                       e[:, s * 512:(s + 1) * 512],
                       start=(jb == 0), stop=(jb == NB - 1))

        prev = None
        for jb in range(NB):
            simp = do_sim(jb)
            if prev is not None:
                do_av(jb - 1, prev)
            prev = do_e(jb, simp)
        do_av(NB - 1, prev)

    def sim_exp_1(h, p):
        """Attend1 S-stage: sims -> exp -> mask into 16 resident e tiles.
        Emitted one pass ahead so the PE has independent work during the
        previous pass's normalization chain."""
        hh = 64 * (h % 2)
        k1h = k1T_sb[h // 2][hh:hh + 64, :]
        qh = qT_sb[h // 2][hh:hh + 64, p * PASS:(p + 1) * PASS]
        es = []
        for jb in range(NB):
            simp = ps_a.tile([128, PASS], FP32, tag="sim", name="sim")
            for col in (0, 512):
                mm(simp[:, col:col + 512],
                   k1h[:, jb * 128:(jb + 1) * 128],
                   qh[:, col:col + 512],
                   start=True, stop=True)
            e = e1p.tile([128, PASS], MM_DT, tag=f"e1_{jb}", name=f"e1_{jb}")
            runs, skip = _runs_for(jb, p)
            for (t0, t1) in runs:
                nc.scalar.activation(e[:, t0 * 128:t1 * 128],
                                     simp[:, t0 * 128:t1 * 128], ACT.Exp)
            if skip is not None:
                nc.vector.memset(e[:, skip[0] * 128:skip[1] * 128], 0.0)
            td = jb - 8 * p
            if 0 <= td < 8:
                blk = e[:, td * 128:(td + 1) * 128]
                nc.vector.tensor_mul(blk, blk, tri_le[:])
            ta = jb - 4 - 8 * p
            if 0 <= ta < 8:
                blk = e[:, ta * 128:(ta + 1) * 128]
                nc.vector.tensor_mul(blk, blk, tri_gt[:])
            es.append(e)
        return es

    def wout_half(p):
        """Phase 3 for the column half finished by pass group p."""
        for nb in range(8 * p, 8 * p + 8):
            pool, tag = (ps_b, "av") if nb % 2 == 0 else (ps_a, "sim")
            acc = pool.tile([128, PASS], FP32, tag=tag, name=tag)
            for s in range(2):
                for kt in range(2):
                    mm(acc[:, s * 512:(s + 1) * 512],
                       o2T[kt][:, nb * 128:(nb + 1) * 128],
                       wout_sb[kt][:, s * 512:(s + 1) * 512],
                       start=(kt == 0), stop=(kt == 1))
            osb = osb_p.tile([128, DQ], FP32, tag="osb", name="osb")
            if nb % 2 == 0:
                nc.vector.tensor_copy(osb[:], acc[:])
            else:
                nc.scalar.copy(osb[:], acc[:])
            nc.sync.dma_start(out=out[nb * 128:(nb + 1) * 128, :], in_=osb[:])

    passes = [(h, p) for p in range(2) for h in range(HEADS)]
    e1s = sim_exp_1(*passes[0])
    for idx, (h, p) in enumerate(passes):
        hh = 64 * (h % 2)

        # ------------- attend 1 V-stage: av + denominator matmuls ---------
        out1 = ps_b.tile([128, PASS], FP32, tag="av", name="av")
        ones = ps_on.tile([33, 512], FP32, tag="ones", name="ones")
        for jb in range(NB):
            for s in range(2):
                mm(out1[:, s * 512:(s + 1) * 512],
                   v1_sb[jb][:, 128 * h:128 * h + 128],
                   e1s[jb][:, s * 512:(s + 1) * 512],
                   start=(jb == 0), stop=(jb == NB - 1))
            for s in range(2):
                mm(ones[32 * s:32 * s + 1, :], onescol[:],
                   e1s[jb][:, s * 512:(s + 1) * 512],
                   start=(jb == 0), stop=(jb == NB - 1))

        # normalize (z = out1 / denom) + silu -> hT
        zf = npool.tile([128, PASS], FP32, tag="z", name="z")
        rbs = npool.tile([128, PASS], FP32, tag="rb", name="rb")
        for s_ in range(2):
            ds_ = npool.tile([1, PASS], FP32, tag="ds", name="ds")
            nc.vector.tensor_copy(ds_[0:1, 0:512], ones[32 * s_:32 * s_ + 1, :])
            nc.vector.tensor_scalar_add(ds_[0:1, 0:512], ds_[0:1, 0:512],
                                        esink[0:1, h:h + 1])
            nc.vector.reciprocal_approx_fast(ds_[0:1, 0:512], ds_[0:1, 0:512])
            rbp = ps_bc.tile([128, 512], FP32, tag="bc", name="bc")
            mm(rbp[:], onesrow[:], ds_[0:1, 0:512], start=True, stop=True)
            nc.scalar.copy(rbs[:, s_ * 512:(s_ + 1) * 512], rbp[:])
        nc.vector.tensor_mul(zf[:], out1[:], rbs[:])
        # silu(z) = z * sigmoid(z) = z / (1 + exp(-z)); stays in the
        # Exp activation table (Silu lives in a different table)
        tql = npool.tile([128, PASS], FP32, tag="tq", name="tq")
        nc.scalar.activation(tql[:], zf[:], ACT.Exp, scale=-1.0)
        nc.vector.tensor_scalar_add(tql[:], tql[:], 1.0)
        nc.vector.reciprocal_approx_fast(tql[:], tql[:])
        hT = npool.tile([128, PASS], MM_DT, tag="hT", name="hT")
        nc.vector.tensor_mul(hT[:], zf[:], tql[:])

        # next pass's S-stage: fills the PE while the chain above runs
        if idx + 1 < len(passes):
            e1s = sim_exp_1(*passes[idx + 1])

        # ------------- attend 2 (fused jb-pipelined) -------------
        k2h = k2T_sb[h][:]
        out2 = ps_b.tile([65, PASS], FP32, tag="av", name="av")
        masked_exp_av(
            k2h, hT[:], lambda jb: v2a_sb[jb][:, 65 * h:65 * h + 65],
            out2[:], None, p)

        # normalize attend2 (denominator rode along as row 64)
        d2 = npool.tile([1, PASS], FP32, tag="ds", name="ds")
        nc.vector.tensor_copy(d2[:], out2[64:65, :])
        nc.vector.tensor_scalar_add(d2[:], d2[:], esink[0:1, h:h + 1])
        nc.vector.reciprocal_approx_fast(d2[:], d2[:])
        rbs2 = npool.tile([64, PASS], FP32, tag="rb2", name="rb2")
        for s_ in range(2):
            rbp = ps_bc.tile([128, 512], FP32, tag="bc", name="bc")
            mm(rbp[0:64, :], onesrow[0:1, 0:64],
               d2[0:1, s_ * 512:(s_ + 1) * 512], start=True, stop=True)
            nc.scalar.copy(rbs2[:, s_ * 512:(s_ + 1) * 512], rbp[0:64, :])
        dst = o2T[h // 2][hh:hh + 64, p * PASS:(p + 1) * PASS]
        nc.vector.tensor_mul(dst, out2[0:64, :], rbs2[:])

        if DEBUG and h == 0 and p == 0:
            nc.sync.dma_start(out=io["dbg_hT"].bitcast(MM_DT), in_=hT[:])
            dzf = npool.tile([128, PASS], FP32, tag="dzf", name="dzf")
            nc.vector.tensor_copy(dzf[:], zf[:])
            nc.sync.dma_start(out=io["dbg_zf"], in_=dzf[:])
            do2 = npool.tile([65, PASS], FP32, tag="do2", name="do2")
            nc.vector.tensor_copy(do2[:], out2[:])
            nc.sync.dma_start(out=io["dbg_out2"], in_=do2[:])

        # interleave the output projection for the completed column half
        if idx == len(passes) - 1 or (idx + 1 < len(passes)
                                      and passes[idx + 1][1] != p):
            wout_half(p)

    if DEBUG:
        for t in range(2):
            nc.sync.dma_start(out=io["dbg_qT"][t * 128:(t + 1) * 128, :].bitcast(MM_DT),
                              in_=qT_sb[t][:])
            nc.sync.dma_start(out=io["dbg_k1T"][t * 128:(t + 1) * 128, :].bitcast(MM_DT),
                              in_=k1T_sb[t][:])
            nc.sync.dma_start(out=io["dbg_o2T"][t * 128:(t + 1) * 128, :].bitcast(MM_DT),
                              in_=o2T[t][:])
        for t in range(4):
            nc.sync.dma_start(out=io["dbg_v1"][t * 128:(t + 1) * 128, :].bitcast(MM_DT),
                              in_=v1_sb[t][:])

    for p_ in reversed(_pools2):
        p_.release()
    for p_ in (stat, const):
        p_.release()


_NC_CACHE = {}


def build_nc():
    key = (str(MM_DT), REPS, DEBUG, PROJ_ONLY)
    if key in _NC_CACHE:
        return _NC_CACHE[key]
    nc = bacc.Bacc("TRN2", target_bir_lowering=False, debug=False,
                   num_devices=N_CORES)
    io = {
        "xq": nc.dram_tensor("xq", [N, DQ], MM_DT, kind="ExternalInput").ap(),
        "xkv": nc.dram_tensor("xkv", [N, DQ], MM_DT, kind="ExternalInput").ap(),
        "wq": nc.dram_tensor("wq", [DQ, 256], MM_DT, kind="ExternalInput").ap(),
        "wk1": nc.dram_tensor("wk1", [DQ, 256], MM_DT, kind="ExternalInput").ap(),
        "wv1": nc.dram_tensor("wv1", [DQ, 512], MM_DT, kind="ExternalInput").ap(),
        "wk2": nc.dram_tensor("wk2", [DQ, 512], MM_DT, kind="ExternalInput").ap(),
        "wv2": nc.dram_tensor("wv2", [DQ, 256], MM_DT, kind="ExternalInput").ap(),
        "wout": nc.dram_tensor("wout", [256, DQ], MM_DT, kind="ExternalInput").ap(),
        "sink": nc.dram_tensor("sink", [1, HEADS], FP32, kind="ExternalInput").ap(),
        "out": nc.dram_tensor("out", [N, DQ], FP32, kind="ExternalOutput").ap(),
    }
    if DEBUG:
        for nm, shp, dt in (("dbg_qT", [256, N], FP32), ("dbg_k1T", [256, N], FP32),
                            ("dbg_o2T", [256, N], FP32), ("dbg_v1", [512, 512], FP32),
                            ("dbg_hT", [128, PASS], FP32), ("dbg_zf", [128, PASS], FP32),
                            ("dbg_out2", [65, PASS], FP32)):
            shp2 = list(shp)
            if dt is FP32 and nm in ("dbg_qT", "dbg_k1T", "dbg_o2T", "dbg_v1", "dbg_hT"):
                shp2[-1] = shp[-1] // 2   # bf16 payload bitcast into fp32 words
            io[nm] = nc.dram_tensor(nm, shp2, FP32, kind="ExternalOutput").ap()
    if REPS == 0:
        # extra input so the I/O-only program's jax trace-cache key differs
        # from the real kernel's (the cache ignores the BIR payload)
        io["dummy0"] = nc.dram_tensor("dummy0", [1, 8], FP32,
                                      kind="ExternalInput").ap()
    with TileContext(nc) as tc:
        if REPS == 0:
            pool0 = tc.alloc_tile_pool(name="p0", bufs=1)
            t0_ = pool0.tile([128, DQ], MM_DT, name="t0_")
            nc.sync.dma_start(out=t0_[:], in_=io["xq"][0:128, :])
            o0_ = pool0.tile([128, DQ], FP32, name="o0_")
            nc.vector.tensor_copy(o0_[:], t0_[:])
            for nb in range(NB):
                nc.sync.dma_start(out=io["out"][nb * 128:(nb + 1) * 128, :],
                                  in_=o0_[:])
            pool0.release()
        for _ in range(REPS):
            build_kernel(nc, tc, io)
    nc.compile()
    _NC_CACHE[key] = (nc, io)
    return nc, io


_BF16 = None


def _bf16():
    global _BF16
    if _BF16 is None:
        import ml_dtypes
        _BF16 = np.dtype(ml_dtypes.bfloat16)
    return _BF16


def make_in_maps(inputs):
    bf = _bf16()
    xq_b = [np.ascontiguousarray(inputs["queries_input"][b]).astype(bf)
            for b in range(2)]
    xkv_b = [np.ascontiguousarray(inputs["key_values_input"][b]).astype(bf)
             for b in range(2)]
    in_maps = []
    for c in range(N_CORES):
        b, g = c // 4, c % 4
        s64 = slice(g * 256, (g + 1) * 256)
        s128 = slice(g * 512, (g + 1) * 512)
        in_maps.append({
            "xq": xq_b[b],
            "xkv": xkv_b[b],
            "wq": np.ascontiguousarray(inputs["Wq"][:, s64]).astype(bf),
            "wk1": np.ascontiguousarray(inputs["Wk1"][:, s64]).astype(bf),
            "wv1": np.ascontiguousarray(inputs["Wv1"][:, s128]).astype(bf),
            "wk2": np.ascontiguousarray(inputs["Wk2"][:, s128]).astype(bf),
            "wv2": np.ascontiguousarray(inputs["Wv2"][:, s64]).astype(bf),
            "wout": np.ascontiguousarray(inputs["Wout"][s64, :]).astype(bf),
            "sink": np.ascontiguousarray(
                inputs["attn_sink"][g * 4:(g + 1) * 4]).reshape(1, HEADS)
                .astype(np.float32),
        })
    return in_maps


def kernel(**inputs):
    from concourse.bass_utils import run_bass_kernel_spmd

    inputs = {k: np.asarray(v) for k, v in inputs.items()}
    nc, _ = build_nc()
    in_maps = make_in_maps(inputs)
    res = run_bass_kernel_spmd(nc, in_maps, list(range(N_CORES)))
    out = np.zeros((2, N, DQ), dtype=np.float32)
    for c in range(N_CORES):
        out[c // 4] += res.results[c]["out"]
    return out


# revision 35
# speedup vs baseline: 794.8969x; 1.0512x over previous
"""Trainium2 Bass kernel for nn_Attention_31997506355363 (sparse_attention).

Sharding: 8 cores = 2 batches x 4 head-groups (4 heads of 16 each).
Each core computes its batch's full-sequence double-attend for its 4 heads,
plus the partial output projection (Wout rows for its heads); host sums the
4 head-group partials per batch.

Math notes (verified vs reference):
  - mask keeps j<=i OR j>i+512  (the strip i<j<=i+512 is masked out)
  - softmax has a per-head sink logit in the denominator only
  - |sim| <= ~6.4 so softmax runs without max-subtraction: p = exp(sim),
    denom = sum_j p + exp(sink)
  - attends are computed transposed: simT[j,i] tiles -> exp -> outT
    accumulated as v.T @ p per 128-j-block (contraction always on the
    partition dim, so no attention-matrix transposes are needed, and
    attend1's output hiddensT feeds attend2 directly)

Perf structure (v2):
  - all matmul operands bf16 (fp32 PE runs at 1/4 rate; tolerance is 2e-2)
  - x transposed by XBAR DMA-transpose (2-byte dtype) straight into SBUF;
    no PE transposes, no PSUM->SBUF copies for xT
  - everything SBUF-resident between phases; weights loaded once;
    phase-1-only pools (xT, projection weights, wide PSUM accs) released
    before the attends
  - projections run stationary-major (one Ldweights per (w-slice), 4
    full-width moving matmuls) to cut PE sequencer pressure
  - masking via DVE multiplies with constant 0/1 triangular tiles + DVE
    memsets; GPSIMD only does one-time constant setup
  - softmax denominators: ones-row matmuls accumulate alongside v.T @ e;
    reciprocal broadcast back to 128 partitions via a rank-1 PE matmul
"""

import sys

for _p in ("/opt/trn_rl_repo",):
    if _p not in sys.path:
        sys.path.insert(0, _p)

import numpy as np
import concourse.bass as bass
from concourse import bacc
import concourse.mybir as mybir
from concourse.tile import TileContext
from concourse.masks import make_identity

FP32 = mybir.dt.float32
MM_DT = mybir.dt.bfloat16
N_CORES = 8
N = 2048            # sequence length
DQ = 1024           # model dim
HEADS = 4           # heads per core
SCALE = 0.125       # 64 ** -0.5, folded into k1T / k2T at projection copy
NB = N // 128       # 16 key blocks
PASS = 1024         # attend i-pass width (2 passes)
ACT = mybir.ActivationFunctionType

DEBUG = False
REPS = 1
PROJ_ONLY = False   # timing experiment: stop after projections


def _runs_for(jb, p):
    """i-subblock runs (in 128-col units within a 1024-wide pass) that are
    not fully masked for key-block jb.  Sub-block t covers queries
    I = 8p + t; (I, jb) is fully masked iff 1 <= jb - I <= 3."""
    skip_lo = max(0, jb - 8 * p - 3)
    skip_hi = min(8, jb - 8 * p)
    if skip_lo >= skip_hi:
        return [(0, 8)], None
    runs = []
    if skip_lo > 0:
        runs.append((0, skip_lo))
    if skip_hi < 8:
        runs.append((skip_hi, 8))
    return runs, (skip_lo, skip_hi)


def build_kernel(nc, tc, io):
    mm = nc.tensor.matmul

    xq, xkv = io["xq"], io["xkv"]
    wq, wk1, wv1, wk2, wv2, wout, sink = (
        io["wq"], io["wk1"], io["wv1"], io["wk2"], io["wv2"], io["wout"],
        io["sink"],
    )
    out = io["out"]

    const = tc.alloc_tile_pool(name="const", bufs=1)
    stat = tc.alloc_tile_pool(name="stat", bufs=1)
    # phase-1-only pools (released before the attends)
    xt_p = tc.alloc_tile_pool(name="xt", bufs=1)
    xin = tc.alloc_tile_pool(name="xin", bufs=1)
    wpool = tc.alloc_tile_pool(name="w", bufs=1)
    ps_w = tc.alloc_tile_pool(name="ps_w", bufs=2, space="PSUM")   # 4 banks
    ps_tp = tc.alloc_tile_pool(name="ps_tp", bufs=2, space="PSUM")  # 2 banks

    ident = const.tile([128, 128], MM_DT, tag="ident", name="ident")
    make_identity(nc, ident[:])

    # ---- constants ----
    onescol = const.tile([128, 1], MM_DT, tag="onescol", name="onescol")
    nc.vector.memset(onescol[:], 1.0)
    onesrow = const.tile([1, 128], FP32, tag="onesrow", name="onesrow")
    nc.vector.memset(onesrow[:], 1.0)
    ones4 = const.tile([128, HEADS], MM_DT, tag="ones4", name="ones4")
    nc.vector.memset(ones4[:], 1.0)

    # 0/1 triangular masks (e layout is [j partitions, i cols]):
    # tri_le keeps jj <= ii (diagonal block), tri_gt keeps jj > ii (block I+4)
    tri_le = const.tile([128, 128], MM_DT, tag="tri_le", name="tri_le")
    nc.gpsimd.memset(tri_le[:], 1.0)
    nc.gpsimd.affine_select(
        out=tri_le[:], in_=tri_le[:], compare_op=mybir.AluOpType.is_ge,
        fill=0.0, base=0, pattern=[[1, 128]], channel_multiplier=-1)
    tri_gt = const.tile([128, 128], MM_DT, tag="tri_gt", name="tri_gt")
    nc.gpsimd.memset(tri_gt[:], 1.0)
    nc.gpsimd.affine_select(
        out=tri_gt[:], in_=tri_gt[:], compare_op=mybir.AluOpType.is_ge,
        fill=0.0, base=-1, pattern=[[-1, 128]], channel_multiplier=1)

    # ---- weights (DMAs ordered around the transposes; see below) ----
    def load_w(w_dram, cols, nm, eng):
        wt = [wpool.tile([128, cols], MM_DT, tag=f"{nm}{kt}", name=f"{nm}{kt}")
              for kt in range(8)]
        for kt in range(8):
            e = eng if not isinstance(eng, tuple) else eng[kt % 2]
            e.dma_start(out=wt[kt][:], in_=w_dram[kt * 128:(kt + 1) * 128, :])
        return wt

    wq_sb = load_w(wq, 256, "wq", (nc.sync, nc.scalar))

    # ---- persistent SBUF intermediates ----
    qT_sb = [stat.tile([128, N], MM_DT, tag=f"qT{t}", name=f"qT{t}") for t in range(2)]
    k1T_sb = [stat.tile([128, N], MM_DT, tag=f"k1T{t}", name=f"k1T{t}") for t in range(2)]
    k2T_sb = [stat.tile([128, N], MM_DT, tag=f"k2T{t}", name=f"k2T{t}") for t in range(4)]
    v1_sb = [stat.tile([128, 512], MM_DT, tag=f"v1_{t}", name=f"v1_{t}") for t in range(NB)]
    v2a_sb = [stat.tile([128, 65 * HEADS], MM_DT, tag=f"v2a{t}", name=f"v2a{t}")
              for t in range(NB)]
    o2T = [stat.tile([128, N], MM_DT, tag=f"o2T{t}", name=f"o2T{t}") for t in range(2)]

    # =====================================================================
    # Phase 1: DMA-transpose x into SBUF, then stationary-major projections.
    # =====================================================================
    xqT = [xt_p.tile([128, N], MM_DT, tag=f"xqT{kt}", name=f"xqT{kt}")
           for kt in range(8)]
    xkvT = [xt_p.tile([128, N], MM_DT, tag=f"xkvT{kt}", name=f"xkvT{kt}")
            for kt in range(8)]

    def load_chunk(x_dram, c, qi):
        nat = []
        for nbl in range(4):
            r0 = c * 512 + nbl * 128
            t = xin.tile([128, DQ], MM_DT, tag=f"x{qi}{nbl}", name=f"x{qi}{nbl}")
            eng = nc.sync if (nbl % 2 == 0) else nc.scalar
            eng.dma_start(out=t[:], in_=x_dram[r0:r0 + 128, :])
            nat.append(t)
        return nat

    def transpose_nat(nat, xT, c):
        """PE-transpose a loaded 512-row chunk into xT[kt][:, c-cols].
        (The XBAR DMA-transpose path raced with compute consumers on HW —
        its completion semaphore does not reliably gate reads.)"""
        for kt in range(8):
            ps = ps_tp.tile([128, 512], MM_DT, tag="tp", name="tp")
            for nbl in range(4):
                nc.tensor.transpose(
                    ps[:, nbl * 128:(nbl + 1) * 128],
                    nat[nbl][:, kt * 128:(kt + 1) * 128], ident[:])
            if kt % 2 == 0:
                nc.vector.tensor_copy(xT[kt][:, c * 512:(c + 1) * 512], ps[:])
            else:
                nc.scalar.copy(xT[kt][:, c * 512:(c + 1) * 512], ps[:])

    def load_rest_of_weights():
        # emitted after the first chunk's x loads so the PE isn't starved
        # at startup waiting for transposable data behind 40 weight DMAs
        w = {}
        w["k1"] = load_w(wk1, 256, "wk1", nc.sync)
        w["k2"] = load_w(wk2, 512, "wk2", nc.scalar)
        w["v1"] = load_w(wv1, 512, "wv1", nc.sync)
        w["v2"] = load_w(wv2, 256, "wv2", nc.scalar)
        w["out"] = [stat.tile([128, DQ], MM_DT, tag=f"wo{t}", name=f"wo{t}")
                    for t in range(2)]
        for t in range(2):
            nc.scalar.dma_start(out=w["out"][t][:],
                                in_=wout[t * 128:(t + 1) * 128, :])
        sink_sb = const.tile([1, HEADS], FP32, tag="sink", name="sink")
        nc.scalar.dma_start(out=sink_sb[:], in_=sink[:])
        esink = const.tile([1, HEADS], FP32, tag="esink", name="esink")
        nc.scalar.activation(esink[:], sink_sb[:], ACT.Exp)
        return w, esink

    # q/k1/k2 groups: stationary-major (one Ldweights per (w-slice, kt, half),
    # two 512-wide moving matmuls); v1+v2 fused on a shared stationary.
    def proj_groups(hf):
        groups = (
            [(qT_sb[m], wq_sb, m, xqT, None) for m in range(2)]
            + [(k1T_sb[m], wk1_sb, m, xkvT, SCALE) for m in range(2)]
            + [(k2T_sb[m], wk2_sb, m, xkvT, SCALE) for m in range(4)]
        )
        cols = slice(hf * 1024, (hf + 1) * 1024)
        for gi, (dst, wsb, m, xT, scale) in enumerate(groups):
            acc = ps_w.tile([128, PASS], FP32, tag="pw", name="pw")
            for kt in range(8):
                for cb in range(2):
                    c0 = hf * 1024 + cb * 512
                    mm(acc[:, cb * 512:(cb + 1) * 512],
                       wsb[kt][:, m * 128:(m + 1) * 128],
                       xT[kt][:, c0:c0 + 512],
                       start=(kt == 0), stop=(kt == 7))
            if scale is None:
                if gi % 2 == 0:
                    nc.vector.tensor_copy(dst[:, cols], acc[:])
                else:
                    nc.scalar.copy(dst[:, cols], acc[:])
            else:
                if gi % 2 == 0:
                    nc.vector.tensor_scalar_mul(dst[:, cols], acc[:], scale)
                else:
                    nc.scalar.mul(dst[:, cols], acc[:], scale)

    def proj_v(hf):
        for nb in range(8 * hf, 8 * hf + 8):
            acc = ps_w.tile([128, PASS], FP32, tag="pw", name="pw")
            for kt in range(8):
                mm(acc[:, 0:512], xkvT[kt][:, nb * 128:(nb + 1) * 128], wv1_sb[kt][:],
                   start=(kt == 0), stop=(kt == 7))
                mm(acc[:, 512:768], xkvT[kt][:, nb * 128:(nb + 1) * 128], wv2_sb[kt][:],
                   start=(kt == 0), stop=(kt == 7))
            if nb % 2 == 0:
                nc.vector.tensor_copy(v1_sb[nb][:], acc[:, 0:512])
            else:
                nc.scalar.copy(v1_sb[nb][:], acc[:, 0:512])
            # pack v2 [h*64 cols] into 65-col groups with a ones column
            sv = v2a_sb[nb][:].rearrange("p (h c) -> p h c", h=HEADS)
            nc.vector.tensor_copy(
                sv[:, :, 0:64],
                acc[:, 512:768].rearrange("p (h c) -> p h c", h=HEADS))
            nc.vector.tensor_copy(
                sv[:, :, 64:65],
                ones4[:].rearrange("p (h c) -> p h c", h=HEADS))

    natq0 = load_chunk(xq, 0, "q")
    natk0 = load_chunk(xkv, 0, "k")
    natq1 = load_chunk(xq, 1, "q2")
    natk1 = load_chunk(xkv, 1, "k2")
    transpose_nat(natq0, xqT, 0)
    transpose_nat(natk0, xkvT, 0)
    _w, esink = load_rest_of_weights()
    wk1_sb, wk2_sb, wv1_sb, wv2_sb, wout_sb = (
        _w["k1"], _w["k2"], _w["v1"], _w["v2"], _w["out"])
    transpose_nat(natq1, xqT, 1)
    transpose_nat(natk1, xkvT, 1)
    natq2 = load_chunk(xq, 2, "q")
    natk2 = load_chunk(xkv, 2, "k")
    natq3 = load_chunk(xq, 3, "q2")
    natk3 = load_chunk(xkv, 3, "k2")
    proj_groups(0)
    proj_v(0)
    transpose_nat(natq2, xqT, 2)
    transpose_nat(natk2, xkvT, 2)
    transpose_nat(natq3, xqT, 3)
    transpose_nat(natk3, xkvT, 3)
    proj_groups(1)
    proj_v(1)

    ps_tp.release()
    ps_w.release()
    wpool.release()
    xin.release()
    xt_p.release()

    # attend-phase pools (allocated after the phase-1 pools are released)
    e1p = tc.alloc_tile_pool(name="e1", bufs=1)    # 16 resident e tiles
    epool = tc.alloc_tile_pool(name="e", bufs=3)
    npool = tc.alloc_tile_pool(name="nrm", bufs=2)
    osb_p = tc.alloc_tile_pool(name="osb", bufs=2)
    ps_a = tc.alloc_tile_pool(name="ps_a", bufs=2, space="PSUM")   # 4 banks
    ps_b = tc.alloc_tile_pool(name="ps_b", bufs=1, space="PSUM")   # 2 banks
    ps_on = tc.alloc_tile_pool(name="ps_on", bufs=1, space="PSUM")  # 1 bank
    ps_bc = tc.alloc_tile_pool(name="ps_bc", bufs=1, space="PSUM")  # 1 bank
    _pools2 = [e1p, epool, npool, osb_p, ps_a, ps_b, ps_on, ps_bc]

    if PROJ_ONLY:
        for nb in range(NB):
            osb = osb_p.tile([128, DQ], FP32, tag="osb", name="osb")
            nc.vector.tensor_copy(osb[:, 0:512], v1_sb[nb][:])
            nc.vector.tensor_copy(osb[:, 512:1024], v1_sb[nb][:])
            nc.sync.dma_start(out=out[nb * 128:(nb + 1) * 128, :], in_=osb[:])
        for p_ in reversed(_pools2):
            p_.release()
        for p_ in (stat, const):
            p_.release()
        return

    # =====================================================================
    # Phase 2: attends (everything SBUF-resident)
    # =====================================================================
    def masked_exp_av(k_h, rhs_h, v_ap, out_ps, ones_ps, p):
        """One attend pass: for each key block jb, sim -> exp -> mask ->
        accumulate v.T @ e (and the ones row for attend1 denominators).

        Software-pipelined one jb deep: the PE emission order is
        sim(0), sim(1), av(0), sim(2), av(1), ... so the in-order PE queue
        never stalls on exp/mask of the block it is about to accumulate."""
        def do_sim(jb):
            simp = ps_a.tile([128, PASS], FP32, tag="sim", name="sim")
            for col in (0, 512):
                mm(simp[:, col:col + 512],
                   k_h[:, jb * 128:(jb + 1) * 128],
                   rhs_h[:, col:col + 512],
                   start=True, stop=True)
            return simp

        def do_e(jb, simp):
            runs, skip = _runs_for(jb, p)
            e = epool.tile([128, PASS], MM_DT, tag="e", name="e")
            for (t0, t1) in runs:
                nc.scalar.activation(e[:, t0 * 128:t1 * 128],
                                     simp[:, t0 * 128:t1 * 128], ACT.Exp)
            if skip is not None:
                nc.vector.memset(e[:, skip[0] * 128:skip[1] * 128], 0.0)
            td = jb - 8 * p
            if 0 <= td < 8:   # diagonal block: keep jj <= ii
                blk = e[:, td * 128:(td + 1) * 128]
                nc.vector.tensor_mul(blk, blk, tri_le[:])
            ta = jb - 4 - 8 * p
            if 0 <= ta < 8:   # jb == I+4 block: keep jj > ii
                blk = e[:, ta * 128:(ta + 1) * 128]
                nc.vector.tensor_mul(blk, blk, tri_gt[:])
            return e

        def do_av(jb, e):
            for s in range(2):
                mm(out_ps[:, s * 512:(s + 1) * 512],
                   v_ap(jb),
                   e[:, s * 512:(s + 1) * 512],
                   start=(jb == 0), stop=(jb == NB - 1))
            if ones_ps is not None:
                for s in range(2):
                    mm(ones_ps[32 * s:32 * s + 1, :], onescol[:],
                       e[:, s * 512:(s + 1) * 512],
                       start=(jb == 0), stop=(jb == NB - 1))

        prev = None
        for jb in range(NB):
            simp = do_sim(jb)
            if prev is not None:
                do_av(jb - 1, prev)
            prev = do_e(jb, simp)
        do_av(NB - 1, prev)

    def sim_exp_1(h, p):
        """Attend1 S-stage: sims -> exp -> mask into 16 resident e tiles.
        Emitted one pass ahead so the PE has independent work during the
        previous pass's normalization chain."""
        hh = 64 * (h % 2)
        k1h = k1T_sb[h // 2][hh:hh + 64, :]
        qh = qT_sb[h // 2][hh:hh + 64, p * PASS:(p + 1) * PASS]
        es = []
        for jb in range(NB):
            simp = ps_a.tile([128, PASS], FP32, tag="sim", name="sim")
            for col in (0, 512):
                mm(simp[:, col:col + 512],
                   k1h[:, jb * 128:(jb + 1) * 128],
                   qh[:, col:col + 512],
                   start=True, stop=True)
            e = e1p.tile([128, PASS], MM_DT, tag=f"e1_{jb}", name=f"e1_{jb}")
            runs, skip = _runs_for(jb, p)
            for (t0, t1) in runs:
                nc.scalar.activation(e[:, t0 * 128:t1 * 128],
                                     simp[:, t0 * 128:t1 * 128], ACT.Exp)
            if skip is not None:
                nc.vector.memset(e[:, skip[0] * 128:skip[1] * 128], 0.0)
            td = jb - 8 * p
            if 0 <= td < 8:
                blk = e[:, td * 128:(td + 1) * 128]
                nc.vector.tensor_mul(blk, blk, tri_le[:])
            ta = jb - 4 - 8 * p
            if 0 <= ta < 8:
                blk = e[:, ta * 128:(ta + 1) * 128]
                nc.vector.tensor_mul(blk, blk, tri_gt[:])
            es.append(e)
        return es

    def wout_half(p):
        """Phase 3 for the column half finished by pass group p."""
        for nb in range(8 * p, 8 * p + 8):
            pool, tag = (ps_b, "av") if nb % 2 == 0 else (ps_a, "sim")
            acc = pool.tile([128, PASS], FP32, tag=tag, name=tag)
            for s in range(2):
                for kt in range(2):
                    mm(acc[:, s * 512:(s + 1) * 512],
                       o2T[kt][:, nb * 128:(nb + 1) * 128],
                       wout_sb[kt][:, s * 512:(s + 1) * 512],
                       start=(kt == 0), stop=(kt == 1))
            osb = osb_p.tile([128, DQ], FP32, tag="osb", name="osb")
            if nb % 2 == 0:
                nc.vector.tensor_copy(osb[:], acc[:])
            else:
                nc.scalar.copy(osb[:], acc[:])
            nc.sync.dma_start(out=out[nb * 128:(nb + 1) * 128, :], in_=osb[:])

    passes = [(h, p) for p in range(2) for h in range(HEADS)]
    e1s = sim_exp_1(*passes[0])
    for idx, (h, p) in enumerate(passes):
        hh = 64 * (h % 2)

        # ------------- attend 1 V-stage: av + denominator matmuls ---------
        out1 = ps_b.tile([128, PASS], FP32, tag="av", name="av")
        ones = ps_on.tile([33, 512], FP32, tag="ones", name="ones")
        for jb in range(NB):
            for s in range(2):
                mm(out1[:, s * 512:(s + 1) * 512],
                   v1_sb[jb][:, 128 * h:128 * h + 128],
                   e1s[jb][:, s * 512:(s + 1) * 512],
                   start=(jb == 0), stop=(jb == NB - 1))
            for s in range(2):
                mm(ones[32 * s:32 * s + 1, :], onescol[:],
                   e1s[jb][:, s * 512:(s + 1) * 512],
                   start=(jb == 0), stop=(jb == NB - 1))

        # normalize (z = out1 / denom) + silu -> hT, pipelined per
        # 512-column half: half 1's broadcast/copy overlaps half 0's DVE
        # chain, and attend2's first sim chunk can start on hT[:, 0:512]
        # while half 1 is still in flight.
        # silu(z) = z * sigmoid(z) = z / (1 + exp(-z)); stays in the
        # Exp activation table (Silu lives in a different table)
        zf = npool.tile([128, PASS], FP32, tag="z", name="z")
        rbs = npool.tile([128, PASS], FP32, tag="rb", name="rb")
        tql = npool.tile([128, PASS], FP32, tag="tq", name="tq")
        hT = npool.tile([128, PASS], MM_DT, tag="hT", name="hT")
        for s_ in range(2):
            sl = slice(s_ * 512, (s_ + 1) * 512)
            ds_ = npool.tile([1, PASS], FP32, tag="ds", name="ds")
            nc.vector.tensor_copy(ds_[0:1, 0:512], ones[32 * s_:32 * s_ + 1, :])
            nc.vector.tensor_scalar_add(ds_[0:1, 0:512], ds_[0:1, 0:512],
                                        esink[0:1, h:h + 1])
            nc.vector.reciprocal_approx_fast(ds_[0:1, 0:512], ds_[0:1, 0:512])
            rbp = ps_bc.tile([128, 512], FP32, tag="bc", name="bc")
            mm(rbp[:], onesrow[:], ds_[0:1, 0:512], start=True, stop=True)
            nc.scalar.copy(rbs[:, sl], rbp[:])
            nc.vector.tensor_mul(zf[:, sl], out1[:, sl], rbs[:, sl])
            nc.scalar.activation(tql[:, sl], zf[:, sl], ACT.Exp, scale=-1.0)
            nc.vector.tensor_scalar_add(tql[:, sl], tql[:, sl], 1.0)
            nc.vector.reciprocal_approx_fast(tql[:, sl], tql[:, sl])
            nc.vector.tensor_mul(hT[:, sl], zf[:, sl], tql[:, sl])

        # next pass's S-stage: fills the PE while the chain above runs
        if idx + 1 < len(passes):
            e1s = sim_exp_1(*passes[idx + 1])

        # ------------- attend 2 (fused jb-pipelined) -------------
        k2h = k2T_sb[h][:]
        out2 = ps_b.tile([65, PASS], FP32, tag="av", name="av")
        masked_exp_av(
            k2h, hT[:], lambda jb: v2a_sb[jb][:, 65 * h:65 * h + 65],
            out2[:], None, p)

        # normalize attend2 (denominator rode along as row 64)
        d2 = npool.tile([1, PASS], FP32, tag="ds", name="ds")
        nc.vector.tensor_copy(d2[:], out2[64:65, :])
        nc.vector.tensor_scalar_add(d2[:], d2[:], esink[0:1, h:h + 1])
        nc.vector.reciprocal_approx_fast(d2[:], d2[:])
        rbs2 = npool.tile([64, PASS], FP32, tag="rb2", name="rb2")
        for s_ in range(2):
            rbp = ps_bc.tile([128, 512], FP32, tag="bc", name="bc")
            mm(rbp[0:64, :], onesrow[0:1, 0:64],
               d2[0:1, s_ * 512:(s_ + 1) * 512], start=True, stop=True)
            nc.scalar.copy(rbs2[:, s_ * 512:(s_ + 1) * 512], rbp[0:64, :])
        dst = o2T[h // 2][hh:hh + 64, p * PASS:(p + 1) * PASS]
        nc.vector.tensor_mul(dst, out2[0:64, :], rbs2[:])

        if DEBUG and h == 0 and p == 0:
            nc.sync.dma_start(out=io["dbg_hT"].bitcast(MM_DT), in_=hT[:])
            dzf = npool.tile([128, PASS], FP32, tag="dzf", name="dzf")
            nc.vector.tensor_copy(dzf[:], zf[:])
            nc.sync.dma_start(out=io["dbg_zf"], in_=dzf[:])
            do2 = npool.tile([65, PASS], FP32, tag="do2", name="do2")
            nc.vector.tensor_copy(do2[:], out2[:])
            nc.sync.dma_start(out=io["dbg_out2"], in_=do2[:])

        # interleave the output projection for the completed column half
        if idx == len(passes) - 1 or (idx + 1 < len(passes)
                                      and passes[idx + 1][1] != p):
            wout_half(p)

    if DEBUG:
        for t in range(2):
            nc.sync.dma_start(out=io["dbg_qT"][t * 128:(t + 1) * 128, :].bitcast(MM_DT),
                              in_=qT_sb[t][:])
            nc.sync.dma_start(out=io["dbg_k1T"][t * 128:(t + 1) * 128, :].bitcast(MM_DT),
                              in_=k1T_sb[t][:])
            nc.sync.dma_start(out=io["dbg_o2T"][t * 128:(t + 1) * 128, :].bitcast(MM_DT),
                              in_=o2T[t][:])
        for t in range(4):
            nc.sync.dma_start(out=io["dbg_v1"][t * 128:(t + 1) * 128, :].bitcast(MM_DT),
                              in_=v1_sb[t][:])

    for p_ in reversed(_pools2):
        p_.release()
    for p_ in (stat, const):
        p_.release()


_NC_CACHE = {}


def build_nc():
    key = (str(MM_DT), REPS, DEBUG, PROJ_ONLY)
    if key in _NC_CACHE:
        return _NC_CACHE[key]
    nc = bacc.Bacc("TRN2", target_bir_lowering=False, debug=False,
                   num_devices=N_CORES)
    io = {
        "xq": nc.dram_tensor("xq", [N, DQ], MM_DT, kind="ExternalInput").ap(),
        "xkv": nc.dram_tensor("xkv", [N, DQ], MM_DT, kind="ExternalInput").ap(),
        "wq": nc.dram_tensor("wq", [DQ, 256], MM_DT, kind="ExternalInput").ap(),
        "wk1": nc.dram_tensor("wk1", [DQ, 256], MM_DT, kind="ExternalInput").ap(),
        "wv1": nc.dram_tensor("wv1", [DQ, 512], MM_DT, kind="ExternalInput").ap(),
        "wk2": nc.dram_tensor("wk2", [DQ, 512], MM_DT, kind="ExternalInput").ap(),
        "wv2": nc.dram_tensor("wv2", [DQ, 256], MM_DT, kind="ExternalInput").ap(),
        "wout": nc.dram_tensor("wout", [256, DQ], MM_DT, kind="ExternalInput").ap(),
        "sink": nc.dram_tensor("sink", [1, HEADS], FP32, kind="ExternalInput").ap(),
        "out": nc.dram_tensor("out", [N, DQ], FP32, kind="ExternalOutput").ap(),
    }
    if DEBUG:
        for nm, shp, dt in (("dbg_qT", [256, N], FP32), ("dbg_k1T", [256, N], FP32),
                            ("dbg_o2T", [256, N], FP32), ("dbg_v1", [512, 512], FP32),
                            ("dbg_hT", [128, PASS], FP32), ("dbg_zf", [128, PASS], FP32),
                            ("dbg_out2", [65, PASS], FP32)):
            shp2 = list(shp)
            if dt is FP32 and nm in ("dbg_qT", "dbg_k1T", "dbg_o2T", "dbg_v1", "dbg_hT"):
                shp2[-1] = shp[-1] // 2   # bf16 payload bitcast into fp32 words
            io[nm] = nc.dram_tensor(nm, shp2, FP32, kind="ExternalOutput").ap()
    if REPS == 0:
        # extra input so the I/O-only program's jax trace-cache key differs
        # from the real kernel's (the cache ignores the BIR payload)
        io["dummy0"] = nc.dram_tensor("dummy0", [1, 8], FP32,
                                      kind="ExternalInput").ap()
    with TileContext(nc) as tc:
        if REPS == 0:
            pool0 = tc.alloc_tile_pool(name="p0", bufs=1)
            t0_ = pool0.tile([128, DQ], MM_DT, name="t0_")
            nc.sync.dma_start(out=t0_[:], in_=io["xq"][0:128, :])
            o0_ = pool0.tile([128, DQ], FP32, name="o0_")
            nc.vector.tensor_copy(o0_[:], t0_[:])
            for nb in range(NB):
                nc.sync.dma_start(out=io["out"][nb * 128:(nb + 1) * 128, :],
                                  in_=o0_[:])
            pool0.release()
        for _ in range(REPS):
            build_kernel(nc, tc, io)
    nc.compile()
    _NC_CACHE[key] = (nc, io)
    return nc, io


_BF16 = None


def _bf16():
    global _BF16
    if _BF16 is None:
        import ml_dtypes
        _BF16 = np.dtype(ml_dtypes.bfloat16)
    return _BF16


def make_in_maps(inputs):
    bf = _bf16()
    xq_b = [np.ascontiguousarray(inputs["queries_input"][b]).astype(bf)
            for b in range(2)]
    xkv_b = [np.ascontiguousarray(inputs["key_values_input"][b]).astype(bf)
             for b in range(2)]
    in_maps = []
    for c in range(N_CORES):
        b, g = c // 4, c % 4
        s64 = slice(g * 256, (g + 1) * 256)
        s128 = slice(g * 512, (g + 1) * 512)
        in_maps.append({
            "xq": xq_b[b],
            "xkv": xkv_b[b],
            "wq": np.ascontiguousarray(inputs["Wq"][:, s64]).astype(bf),
            "wk1": np.ascontiguousarray(inputs["Wk1"][:, s64]).astype(bf),
            "wv1": np.ascontiguousarray(inputs["Wv1"][:, s128]).astype(bf),
            "wk2": np.ascontiguousarray(inputs["Wk2"][:, s128]).astype(bf),
            "wv2": np.ascontiguousarray(inputs["Wv2"][:, s64]).astype(bf),
            "wout": np.ascontiguousarray(inputs["Wout"][s64, :]).astype(bf),
            "sink": np.ascontiguousarray(
                inputs["attn_sink"][g * 4:(g + 1) * 4]).reshape(1, HEADS)
                .astype(np.float32),
        })
    return in_maps


def kernel(**inputs):
    from concourse.bass_utils import run_bass_kernel_spmd

    inputs = {k: np.asarray(v) for k, v in inputs.items()}
    nc, _ = build_nc()
    in_maps = make_in_maps(inputs)
    res = run_bass_kernel_spmd(nc, in_maps, list(range(N_CORES)))
    out = np.zeros((2, N, DQ), dtype=np.float32)
    for c in range(N_CORES):
        out[c // 4] += res.results[c]["out"]
    return out


# revision 38
# speedup vs baseline: 9259.1689x; 11.6483x over previous
"""Trainium2 Bass kernel for nn_Attention_31997506355363 (sparse_attention).

Sharding: 8 cores = 2 batches x 4 head-groups (4 heads of 16 each).
Each core computes its batch's full-sequence double-attend for its 4 heads,
plus the partial output projection (Wout rows for its heads); host sums the
4 head-group partials per batch.

Math notes (verified vs reference):
  - mask keeps j<=i OR j>i+512  (the strip i<j<=i+512 is masked out)
  - softmax has a per-head sink logit in the denominator only
  - |sim| <= ~6.4 so softmax runs without max-subtraction: p = exp(sim),
    denom = sum_j p + exp(sink)
  - attends are computed transposed: simT[j,i] tiles -> exp -> outT
    accumulated as v.T @ p per 128-j-block (contraction always on the
    partition dim, so no attention-matrix transposes are needed, and
    attend1's output hiddensT feeds attend2 directly)

Perf structure (v2):
  - all matmul operands bf16 (fp32 PE runs at 1/4 rate; tolerance is 2e-2)
  - x transposed by XBAR DMA-transpose (2-byte dtype) straight into SBUF;
    no PE transposes, no PSUM->SBUF copies for xT
  - everything SBUF-resident between phases; weights loaded once;
    phase-1-only pools (xT, projection weights, wide PSUM accs) released
    before the attends
  - projections run stationary-major (one Ldweights per (w-slice), 4
    full-width moving matmuls) to cut PE sequencer pressure
  - masking via DVE multiplies with constant 0/1 triangular tiles + DVE
    memsets; GPSIMD only does one-time constant setup
  - softmax denominators: ones-row matmuls accumulate alongside v.T @ e;
    reciprocal broadcast back to 128 partitions via a rank-1 PE matmul
"""

import sys

for _p in ("/opt/trn_rl_repo",):
    if _p not in sys.path:
        sys.path.insert(0, _p)

import numpy as np
import concourse.bass as bass
from concourse import bacc
import concourse.mybir as mybir
from concourse.tile import TileContext
from concourse.masks import make_identity

FP32 = mybir.dt.float32
MM_DT = mybir.dt.bfloat16
N_CORES = 8
N = 2048            # sequence length
DQ = 1024           # model dim
HEADS = 4           # heads per core
SCALE = 0.125       # 64 ** -0.5, folded into k1T / k2T at projection copy
NB = N // 128       # 16 key blocks
PASS = 1024         # attend i-pass width (2 passes)
ACT = mybir.ActivationFunctionType

DEBUG = False
REPS = 1
PROJ_ONLY = False   # timing experiment: stop after projections


def _runs_for(jb, p):
    """i-subblock runs (in 128-col units within a 1024-wide pass) that are
    not fully masked for key-block jb.  Sub-block t covers queries
    I = 8p + t; (I, jb) is fully masked iff 1 <= jb - I <= 3."""
    skip_lo = max(0, jb - 8 * p - 3)
    skip_hi = min(8, jb - 8 * p)
    if skip_lo >= skip_hi:
        return [(0, 8)], None
    runs = []
    if skip_lo > 0:
        runs.append((0, skip_lo))
    if skip_hi < 8:
        runs.append((skip_hi, 8))
    return runs, (skip_lo, skip_hi)


def build_kernel(nc, tc, io):
    mm = nc.tensor.matmul

    xq, xkv = io["xq"], io["xkv"]
    wq, wk1, wv1, wk2, wv2, wout, sink = (
        io["wq"], io["wk1"], io["wv1"], io["wk2"], io["wv2"], io["wout"],
        io["sink"],
    )
    out = io["out"]

    const = tc.alloc_tile_pool(name="const", bufs=1)
    stat = tc.alloc_tile_pool(name="stat", bufs=1)
    # phase-1-only pools (released before the attends)
    xt_p = tc.alloc_tile_pool(name="xt", bufs=1)
    xin = tc.alloc_tile_pool(name="xin", bufs=1)
    wpool = tc.alloc_tile_pool(name="w", bufs=1)
    ps_w = tc.alloc_tile_pool(name="ps_w", bufs=2, space="PSUM")   # 4 banks
    ps_tp = tc.alloc_tile_pool(name="ps_tp", bufs=2, space="PSUM")  # 2 banks

    ident = const.tile([128, 128], MM_DT, tag="ident", name="ident")
    make_identity(nc, ident[:])

    # ---- constants ----
    onescol = const.tile([128, 1], MM_DT, tag="onescol", name="onescol")
    nc.vector.memset(onescol[:], 1.0)
    onesrow = const.tile([1, 128], FP32, tag="onesrow", name="onesrow")
    nc.vector.memset(onesrow[:], 1.0)
    ones4 = const.tile([128, HEADS], MM_DT, tag="ones4", name="ones4")
    nc.vector.memset(ones4[:], 1.0)

    # 0/1 triangular masks (e layout is [j partitions, i cols]):
    # tri_le keeps jj <= ii (diagonal block), tri_gt keeps jj > ii (block I+4)
    tri_le = const.tile([128, 128], MM_DT, tag="tri_le", name="tri_le")
    nc.gpsimd.memset(tri_le[:], 1.0)
    nc.gpsimd.affine_select(
        out=tri_le[:], in_=tri_le[:], compare_op=mybir.AluOpType.is_ge,
        fill=0.0, base=0, pattern=[[1, 128]], channel_multiplier=-1)
    tri_gt = const.tile([128, 128], MM_DT, tag="tri_gt", name="tri_gt")
    nc.gpsimd.memset(tri_gt[:], 1.0)
    nc.gpsimd.affine_select(
        out=tri_gt[:], in_=tri_gt[:], compare_op=mybir.AluOpType.is_ge,
        fill=0.0, base=-1, pattern=[[-1, 128]], channel_multiplier=1)

    # ---- weights (DMAs ordered around the transposes; see below) ----
    def load_w(w_dram, cols, nm, eng):
        wt = [wpool.tile([128, cols], MM_DT, tag=f"{nm}{kt}", name=f"{nm}{kt}")
              for kt in range(8)]
        for kt in range(8):
            e = eng if not isinstance(eng, tuple) else eng[kt % 2]
            e.dma_start(out=wt[kt][:], in_=w_dram[kt * 128:(kt + 1) * 128, :])
        return wt

    wq_sb = load_w(wq, 256, "wq", (nc.sync, nc.scalar))

    # ---- persistent SBUF intermediates ----
    qT_sb = [stat.tile([128, N], MM_DT, tag=f"qT{t}", name=f"qT{t}") for t in range(2)]
    k1T_sb = [stat.tile([128, N], MM_DT, tag=f"k1T{t}", name=f"k1T{t}") for t in range(2)]
    k2T_sb = [stat.tile([128, N], MM_DT, tag=f"k2T{t}", name=f"k2T{t}") for t in range(4)]
    v1_sb = [stat.tile([128, 512], MM_DT, tag=f"v1_{t}", name=f"v1_{t}") for t in range(NB)]
    v2a_sb = [stat.tile([128, 65 * HEADS], MM_DT, tag=f"v2a{t}", name=f"v2a{t}")
              for t in range(NB)]
    o2T = [stat.tile([128, N], MM_DT, tag=f"o2T{t}", name=f"o2T{t}") for t in range(2)]

    # =====================================================================
    # Phase 1: DMA-transpose x into SBUF, then stationary-major projections.
    # =====================================================================
    xqT = [xt_p.tile([128, N], MM_DT, tag=f"xqT{kt}", name=f"xqT{kt}")
           for kt in range(8)]
    xkvT = [xt_p.tile([128, N], MM_DT, tag=f"xkvT{kt}", name=f"xkvT{kt}")
            for kt in range(8)]

    def load_chunk(x_dram, c, qi):
        nat = []
        for nbl in range(4):
            r0 = c * 512 + nbl * 128
            t = xin.tile([128, DQ], MM_DT, tag=f"x{qi}{nbl}", name=f"x{qi}{nbl}")
            eng = nc.sync if (nbl % 2 == 0) else nc.scalar
            eng.dma_start(out=t[:], in_=x_dram[r0:r0 + 128, :])
            nat.append(t)
        return nat

    def transpose_nat(nat, xT, c):
        """PE-transpose a loaded 512-row chunk into xT[kt][:, c-cols].
        (The XBAR DMA-transpose path raced with compute consumers on HW —
        its completion semaphore does not reliably gate reads.)"""
        for kt in range(8):
            ps = ps_tp.tile([128, 512], MM_DT, tag="tp", name="tp")
            for nbl in range(4):
                nc.tensor.transpose(
                    ps[:, nbl * 128:(nbl + 1) * 128],
                    nat[nbl][:, kt * 128:(kt + 1) * 128], ident[:])
            if kt % 2 == 0:
                nc.vector.tensor_copy(xT[kt][:, c * 512:(c + 1) * 512], ps[:])
            else:
                nc.scalar.copy(xT[kt][:, c * 512:(c + 1) * 512], ps[:])

    def load_rest_of_weights():
        # emitted after the first chunk's x loads so the PE isn't starved
        # at startup waiting for transposable data behind 40 weight DMAs
        w = {}
        w["k1"] = load_w(wk1, 256, "wk1", nc.sync)
        w["k2"] = load_w(wk2, 512, "wk2", nc.scalar)
        w["v1"] = load_w(wv1, 512, "wv1", nc.sync)
        w["v2"] = load_w(wv2, 256, "wv2", nc.scalar)
        w["out"] = [stat.tile([128, DQ], MM_DT, tag=f"wo{t}", name=f"wo{t}")
                    for t in range(2)]
        for t in range(2):
            nc.scalar.dma_start(out=w["out"][t][:],
                                in_=wout[t * 128:(t + 1) * 128, :])
        sink_sb = const.tile([1, HEADS], FP32, tag="sink", name="sink")
        nc.scalar.dma_start(out=sink_sb[:], in_=sink[:])
        esink = const.tile([1, HEADS], FP32, tag="esink", name="esink")
        nc.scalar.activation(esink[:], sink_sb[:], ACT.Exp)
        return w, esink

    # q/k1/k2 groups: stationary-major (one Ldweights per (w-slice, kt, half),
    # two 512-wide moving matmuls); v1+v2 fused on a shared stationary.
    def proj_groups(hf):
        groups = (
            [(qT_sb[m], wq_sb, m, xqT, None) for m in range(2)]
            + [(k1T_sb[m], wk1_sb, m, xkvT, SCALE) for m in range(2)]
            + [(k2T_sb[m], wk2_sb, m, xkvT, SCALE) for m in range(4)]
        )
        cols = slice(hf * 1024, (hf + 1) * 1024)
        for gi, (dst, wsb, m, xT, scale) in enumerate(groups):
            acc = ps_w.tile([128, PASS], FP32, tag="pw", name="pw")
            for kt in range(8):
                for cb in range(2):
                    c0 = hf * 1024 + cb * 512
                    mm(acc[:, cb * 512:(cb + 1) * 512],
                       wsb[kt][:, m * 128:(m + 1) * 128],
                       xT[kt][:, c0:c0 + 512],
                       start=(kt == 0), stop=(kt == 7))
            if scale is None:
                if gi % 2 == 0:
                    nc.vector.tensor_copy(dst[:, cols], acc[:])
                else:
                    nc.scalar.copy(dst[:, cols], acc[:])
            else:
                if gi % 2 == 0:
                    nc.vector.tensor_scalar_mul(dst[:, cols], acc[:], scale)
                else:
                    nc.scalar.mul(dst[:, cols], acc[:], scale)

    def proj_v(hf):
        for nb in range(8 * hf, 8 * hf + 8):
            acc = ps_w.tile([128, PASS], FP32, tag="pw", name="pw")
            for kt in range(8):
                mm(acc[:, 0:512], xkvT[kt][:, nb * 128:(nb + 1) * 128], wv1_sb[kt][:],
                   start=(kt == 0), stop=(kt == 7))
                mm(acc[:, 512:768], xkvT[kt][:, nb * 128:(nb + 1) * 128], wv2_sb[kt][:],
                   start=(kt == 0), stop=(kt == 7))
            if nb % 2 == 0:
                nc.vector.tensor_copy(v1_sb[nb][:], acc[:, 0:512])
            else:
                nc.scalar.copy(v1_sb[nb][:], acc[:, 0:512])
            # pack v2 [h*64 cols] into 65-col groups with a ones column
            sv = v2a_sb[nb][:].rearrange("p (h c) -> p h c", h=HEADS)
            nc.vector.tensor_copy(
                sv[:, :, 0:64],
                acc[:, 512:768].rearrange("p (h c) -> p h c", h=HEADS))
            nc.vector.tensor_copy(
                sv[:, :, 64:65],
                ones4[:].rearrange("p (h c) -> p h c", h=HEADS))

    natq0 = load_chunk(xq, 0, "q")
    natk0 = load_chunk(xkv, 0, "k")
    natq1 = load_chunk(xq, 1, "q2")
    natk1 = load_chunk(xkv, 1, "k2")
    transpose_nat(natq0, xqT, 0)
    transpose_nat(natk0, xkvT, 0)
    transpose_nat(natq1, xqT, 1)
    transpose_nat(natk1, xkvT, 1)
    # weight DMAs AFTER the chunk-1 transposes: the scalar hwdge queue
    # shares the ACT sequencer with the transpose copies, and dispatching
    # 18 weight DMAs first stalls the copies (and the first projection
    # matmul behind them) for ~11us
    _w, esink = load_rest_of_weights()
    wk1_sb, wk2_sb, wv1_sb, wv2_sb, wout_sb = (
        _w["k1"], _w["k2"], _w["v1"], _w["v2"], _w["out"])
    natq2 = load_chunk(xq, 2, "q")
    natk2 = load_chunk(xkv, 2, "k")
    natq3 = load_chunk(xq, 3, "q2")
    natk3 = load_chunk(xkv, 3, "k2")
    proj_groups(0)
    proj_v(0)
    transpose_nat(natq2, xqT, 2)
    transpose_nat(natk2, xkvT, 2)
    transpose_nat(natq3, xqT, 3)
    transpose_nat(natk3, xkvT, 3)
    proj_groups(1)
    proj_v(1)

    ps_tp.release()
    ps_w.release()
    wpool.release()
    xin.release()
    xt_p.release()

    # attend-phase pools (allocated after the phase-1 pools are released)
    e1p = tc.alloc_tile_pool(name="e1", bufs=1)    # 16 resident e tiles
    epool = tc.alloc_tile_pool(name="e", bufs=3)
    npool = tc.alloc_tile_pool(name="nrm", bufs=2)
    osb_p = tc.alloc_tile_pool(name="osb", bufs=2)
    ps_a = tc.alloc_tile_pool(name="ps_a", bufs=2, space="PSUM")   # 4 banks
    ps_b = tc.alloc_tile_pool(name="ps_b", bufs=1, space="PSUM")   # 2 banks
    ps_on = tc.alloc_tile_pool(name="ps_on", bufs=1, space="PSUM")  # 1 bank
    ps_bc = tc.alloc_tile_pool(name="ps_bc", bufs=1, space="PSUM")  # 1 bank
    _pools2 = [e1p, epool, npool, osb_p, ps_a, ps_b, ps_on, ps_bc]

    if PROJ_ONLY:
        for nb in range(NB):
            osb = osb_p.tile([128, DQ], FP32, tag="osb", name="osb")
            nc.vector.tensor_copy(osb[:, 0:512], v1_sb[nb][:])
            nc.vector.tensor_copy(osb[:, 512:1024], v1_sb[nb][:])
            nc.sync.dma_start(out=out[nb * 128:(nb + 1) * 128, :], in_=osb[:])
        for p_ in reversed(_pools2):
            p_.release()
        for p_ in (stat, const):
            p_.release()
        return

    # =====================================================================
    # Phase 2: attends (everything SBUF-resident)
    # =====================================================================
    def masked_exp_av(k_h, rhs_h, v_ap, out_ps, ones_ps, p):
        """One attend pass: for each key block jb, sim -> exp -> mask ->
        accumulate v.T @ e (and the ones row for attend1 denominators).

        Software-pipelined one jb deep: the PE emission order is
        sim(0), sim(1), av(0), sim(2), av(1), ... so the in-order PE queue
        never stalls on exp/mask of the block it is about to accumulate."""
        def do_sim(jb):
            simp = ps_a.tile([128, PASS], FP32, tag="sim", name="sim")
            for col in (0, 512):
                mm(simp[:, col:col + 512],
                   k_h[:, jb * 128:(jb + 1) * 128],
                   rhs_h[:, col:col + 512],
                   start=True, stop=True)
            return simp

        def do_e(jb, simp):
            runs, skip = _runs_for(jb, p)
            e = epool.tile([128, PASS], MM_DT, tag="e", name="e")
            for (t0, t1) in runs:
                nc.scalar.activation(e[:, t0 * 128:t1 * 128],
                                     simp[:, t0 * 128:t1 * 128], ACT.Exp)
            if skip is not None:
                nc.vector.memset(e[:, skip[0] * 128:skip[1] * 128], 0.0)
            td = jb - 8 * p
            if 0 <= td < 8:   # diagonal block: keep jj <= ii
                blk = e[:, td * 128:(td + 1) * 128]
                nc.vector.tensor_mul(blk, blk, tri_le[:])
            ta = jb - 4 - 8 * p
            if 0 <= ta < 8:   # jb == I+4 block: keep jj > ii
                blk = e[:, ta * 128:(ta + 1) * 128]
                nc.vector.tensor_mul(blk, blk, tri_gt[:])
            return e

        def do_av(jb, e):
            for s in range(2):
                mm(out_ps[:, s * 512:(s + 1) * 512],
                   v_ap(jb),
                   e[:, s * 512:(s + 1) * 512],
                   start=(jb == 0), stop=(jb == NB - 1))
            if ones_ps is not None:
                for s in range(2):
                    mm(ones_ps[32 * s:32 * s + 1, :], onescol[:],
                       e[:, s * 512:(s + 1) * 512],
                       start=(jb == 0), stop=(jb == NB - 1))

        prev = None
        for jb in range(NB):
            simp = do_sim(jb)
            if prev is not None:
                do_av(jb - 1, prev)
            prev = do_e(jb, simp)
        do_av(NB - 1, prev)

    def sim_exp_1(h, p):
        """Attend1 S-stage: sims -> exp -> mask into 16 resident e tiles.
        Emitted one pass ahead so the PE has independent work during the
        previous pass's normalization chain."""
        hh = 64 * (h % 2)
        k1h = k1T_sb[h // 2][hh:hh + 64, :]
        qh = qT_sb[h // 2][hh:hh + 64, p * PASS:(p + 1) * PASS]
        es = []
        for jb in range(NB):
            simp = ps_a.tile([128, PASS], FP32, tag="sim", name="sim")
            for col in (0, 512):
                mm(simp[:, col:col + 512],
                   k1h[:, jb * 128:(jb + 1) * 128],
                   qh[:, col:col + 512],
                   start=True, stop=True)
            e = e1p.tile([128, PASS], MM_DT, tag=f"e1_{jb}", name=f"e1_{jb}")
            runs, skip = _runs_for(jb, p)
            for (t0, t1) in runs:
                nc.scalar.activation(e[:, t0 * 128:t1 * 128],
                                     simp[:, t0 * 128:t1 * 128], ACT.Exp)
            if skip is not None:
                nc.vector.memset(e[:, skip[0] * 128:skip[1] * 128], 0.0)
            td = jb - 8 * p
            if 0 <= td < 8:
                blk = e[:, td * 128:(td + 1) * 128]
                nc.vector.tensor_mul(blk, blk, tri_le[:])
            ta = jb - 4 - 8 * p
            if 0 <= ta < 8:
                blk = e[:, ta * 128:(ta + 1) * 128]
                nc.vector.tensor_mul(blk, blk, tri_gt[:])
            es.append(e)
        return es

    def wout_half(p):
        """Phase 3 for the column half finished by pass group p."""
        for nb in range(8 * p, 8 * p + 8):
            pool, tag = (ps_b, "av") if nb % 2 == 0 else (ps_a, "sim")
            acc = pool.tile([128, PASS], FP32, tag=tag, name=tag)
            for s in range(2):
                for kt in range(2):
                    mm(acc[:, s * 512:(s + 1) * 512],
                       o2T[kt][:, nb * 128:(nb + 1) * 128],
                       wout_sb[kt][:, s * 512:(s + 1) * 512],
                       start=(kt == 0), stop=(kt == 1))
            osb = osb_p.tile([128, DQ], FP32, tag="osb", name="osb")
            if nb % 2 == 0:
                nc.vector.tensor_copy(osb[:], acc[:])
            else:
                nc.scalar.copy(osb[:], acc[:])
            nc.sync.dma_start(out=out[nb * 128:(nb + 1) * 128, :], in_=osb[:])

    passes = [(h, p) for p in range(2) for h in range(HEADS)]
    e1s = sim_exp_1(*passes[0])
    for idx, (h, p) in enumerate(passes):
        hh = 64 * (h % 2)

        # ------------- attend 1 V-stage: av + denominator matmuls ---------
        out1 = ps_b.tile([128, PASS], FP32, tag="av", name="av")
        ones = ps_on.tile([33, 512], FP32, tag="ones", name="ones")
        for jb in range(NB):
            for s in range(2):
                mm(out1[:, s * 512:(s + 1) * 512],
                   v1_sb[jb][:, 128 * h:128 * h + 128],
                   e1s[jb][:, s * 512:(s + 1) * 512],
                   start=(jb == 0), stop=(jb == NB - 1))
            for s in range(2):
                mm(ones[32 * s:32 * s + 1, :], onescol[:],
                   e1s[jb][:, s * 512:(s + 1) * 512],
                   start=(jb == 0), stop=(jb == NB - 1))

        # normalize (z = out1 / denom) + silu -> hT, pipelined per
        # 512-column half: half 1's broadcast/copy overlaps half 0's DVE
        # chain, and attend2's first sim chunk can start on hT[:, 0:512]
        # while half 1 is still in flight.
        # silu(z) = z * sigmoid(z) = z / (1 + exp(-z)); stays in the
        # Exp activation table (Silu lives in a different table)
        zf = npool.tile([128, PASS], FP32, tag="z", name="z")
        rbs = npool.tile([128, PASS], FP32, tag="rb", name="rb")
        tql = npool.tile([128, PASS], FP32, tag="tq", name="tq")
        hT = npool.tile([128, PASS], MM_DT, tag="hT", name="hT")
        for s_ in range(2):
            sl = slice(s_ * 512, (s_ + 1) * 512)
            ds_ = npool.tile([1, PASS], FP32, tag="ds", name="ds")
            nc.vector.tensor_copy(ds_[0:1, 0:512], ones[32 * s_:32 * s_ + 1, :])
            nc.vector.tensor_scalar_add(ds_[0:1, 0:512], ds_[0:1, 0:512],
                                        esink[0:1, h:h + 1])
            nc.vector.reciprocal_approx_fast(ds_[0:1, 0:512], ds_[0:1, 0:512])
            rbp = ps_bc.tile([128, 512], FP32, tag="bc", name="bc")
            mm(rbp[:], onesrow[:], ds_[0:1, 0:512], start=True, stop=True)
            nc.scalar.copy(rbs[:, sl], rbp[:])
            nc.vector.tensor_mul(zf[:, sl], out1[:, sl], rbs[:, sl])
            nc.scalar.activation(tql[:, sl], zf[:, sl], ACT.Exp, scale=-1.0)
            nc.vector.tensor_scalar_add(tql[:, sl], tql[:, sl], 1.0)
            nc.vector.reciprocal_approx_fast(tql[:, sl], tql[:, sl])
            nc.vector.tensor_mul(hT[:, sl], zf[:, sl], tql[:, sl])

        # next pass's S-stage: fills the PE while the chain above runs
        if idx + 1 < len(passes):
            e1s = sim_exp_1(*passes[idx + 1])

        # ------------- attend 2 (fused jb-pipelined) -------------
        k2h = k2T_sb[h][:]
        out2 = ps_b.tile([65, PASS], FP32, tag="av", name="av")
        masked_exp_av(
            k2h, hT[:], lambda jb: v2a_sb[jb][:, 65 * h:65 * h + 65],
            out2[:], None, p)

        # normalize attend2 (denominator rode along as row 64)
        d2 = npool.tile([1, PASS], FP32, tag="ds", name="ds")
        nc.vector.tensor_copy(d2[:], out2[64:65, :])
        nc.vector.tensor_scalar_add(d2[:], d2[:], esink[0:1, h:h + 1])
        nc.vector.reciprocal_approx_fast(d2[:], d2[:])
        rbs2 = npool.tile([64, PASS], FP32, tag="rb2", name="rb2")
        for s_ in range(2):
            rbp = ps_bc.tile([128, 512], FP32, tag="bc", name="bc")
            mm(rbp[0:64, :], onesrow[0:1, 0:64],
               d2[0:1, s_ * 512:(s_ + 1) * 512], start=True, stop=True)
            nc.scalar.copy(rbs2[:, s_ * 512:(s_ + 1) * 512], rbp[0:64, :])
        dst = o2T[h // 2][hh:hh + 64, p * PASS:(p + 1) * PASS]
        nc.vector.tensor_mul(dst, out2[0:64, :], rbs2[:])

        if DEBUG and h == 0 and p == 0:
            nc.sync.dma_start(out=io["dbg_hT"].bitcast(MM_DT), in_=hT[:])
            dzf = npool.tile([128, PASS], FP32, tag="dzf", name="dzf")
            nc.vector.tensor_copy(dzf[:], zf[:])
            nc.sync.dma_start(out=io["dbg_zf"], in_=dzf[:])
            do2 = npool.tile([65, PASS], FP32, tag="do2", name="do2")
            nc.vector.tensor_copy(do2[:], out2[:])
            nc.sync.dma_start(out=io["dbg_out2"], in_=do2[:])

        # interleave the output projection for the completed column half
        if idx == len(passes) - 1 or (idx + 1 < len(passes)
                                      and passes[idx + 1][1] != p):
            wout_half(p)

    if DEBUG:
        for t in range(2):
            nc.sync.dma_start(out=io["dbg_qT"][t * 128:(t + 1) * 128, :].bitcast(MM_DT),
                              in_=qT_sb[t][:])
            nc.sync.dma_start(out=io["dbg_k1T"][t * 128:(t + 1) * 128, :].bitcast(MM_DT),
                              in_=k1T_sb[t][:])
            nc.sync.dma_start(out=io["dbg_o2T"][t * 128:(t + 1) * 128, :].bitcast(MM_DT),
                              in_=o2T[t][:])
        for t in range(4):
            nc.sync.dma_start(out=io["dbg_v1"][t * 128:(t + 1) * 128, :].bitcast(MM_DT),
                              in_=v1_sb[t][:])

    for p_ in reversed(_pools2):
        p_.release()
    for p_ in (stat, const):
        p_.release()


_NC_CACHE = {}


def build_nc():
    key = (str(MM_DT), REPS, DEBUG, PROJ_ONLY)
    if key in _NC_CACHE:
        return _NC_CACHE[key]
    nc = bacc.Bacc("TRN2", target_bir_lowering=False, debug=False,
                   num_devices=N_CORES)
    io = {
        "xq": nc.dram_tensor("xq", [N, DQ], MM_DT, kind="ExternalInput").ap(),
        "xkv": nc.dram_tensor("xkv", [N, DQ], MM_DT, kind="ExternalInput").ap(),
        "wq": nc.dram_tensor("wq", [DQ, 256], MM_DT, kind="ExternalInput").ap(),
        "wk1": nc.dram_tensor("wk1", [DQ, 256], MM_DT, kind="ExternalInput").ap(),
        "wv1": nc.dram_tensor("wv1", [DQ, 512], MM_DT, kind="ExternalInput").ap(),
        "wk2": nc.dram_tensor("wk2", [DQ, 512], MM_DT, kind="ExternalInput").ap(),
        "wv2": nc.dram_tensor("wv2", [DQ, 256], MM_DT, kind="ExternalInput").ap(),
        "wout": nc.dram_tensor("wout", [256, DQ], MM_DT, kind="ExternalInput").ap(),
        "sink": nc.dram_tensor("sink", [1, HEADS], FP32, kind="ExternalInput").ap(),
        "out": nc.dram_tensor("out", [N, DQ], FP32, kind="ExternalOutput").ap(),
    }
    if DEBUG:
        for nm, shp, dt in (("dbg_qT", [256, N], FP32), ("dbg_k1T", [256, N], FP32),
                            ("dbg_o2T", [256, N], FP32), ("dbg_v1", [512, 512], FP32),
                            ("dbg_hT", [128, PASS], FP32), ("dbg_zf", [128, PASS], FP32),
                            ("dbg_out2", [65, PASS], FP32)):
            shp2 = list(shp)
            if dt is FP32 and nm in ("dbg_qT", "dbg_k1T", "dbg_o2T", "dbg_v1", "dbg_hT"):
                shp2[-1] = shp[-1] // 2   # bf16 payload bitcast into fp32 words
            io[nm] = nc.dram_tensor(nm, shp2, FP32, kind="ExternalOutput").ap()
    if REPS == 0:
        # extra input so the I/O-only program's jax trace-cache key differs
        # from the real kernel's (the cache ignores the BIR payload)
        io["dummy0"] = nc.dram_tensor("dummy0", [1, 8], FP32,
                                      kind="ExternalInput").ap()
    with TileContext(nc) as tc:
        if REPS == 0:
            pool0 = tc.alloc_tile_pool(name="p0", bufs=1)
            t0_ = pool0.tile([128, DQ], MM_DT, name="t0_")
            nc.sync.dma_start(out=t0_[:], in_=io["xq"][0:128, :])
            o0_ = pool0.tile([128, DQ], FP32, name="o0_")
            nc.vector.tensor_copy(o0_[:], t0_[:])
            for nb in range(NB):
                nc.sync.dma_start(out=io["out"][nb * 128:(nb + 1) * 128, :],
                                  in_=o0_[:])
            pool0.release()
        for _ in range(REPS):
            build_kernel(nc, tc, io)
    nc.compile()
    _NC_CACHE[key] = (nc, io)
    return nc, io


_BF16 = None


def _bf16():
    global _BF16
    if _BF16 is None:
        import ml_dtypes
        _BF16 = np.dtype(ml_dtypes.bfloat16)
    return _BF16


def make_in_maps(inputs):
    bf = _bf16()
    xq_b = [np.ascontiguousarray(inputs["queries_input"][b]).astype(bf)
            for b in range(2)]
    xkv_b = [np.ascontiguousarray(inputs["key_values_input"][b]).astype(bf)
             for b in range(2)]
    in_maps = []
    for c in range(N_CORES):
        b, g = c // 4, c % 4
        s64 = slice(g * 256, (g + 1) * 256)
        s128 = slice(g * 512, (g + 1) * 512)
        in_maps.append({
            "xq": xq_b[b],
            "xkv": xkv_b[b],
            "wq": np.ascontiguousarray(inputs["Wq"][:, s64]).astype(bf),
            "wk1": np.ascontiguousarray(inputs["Wk1"][:, s64]).astype(bf),
            "wv1": np.ascontiguousarray(inputs["Wv1"][:, s128]).astype(bf),
            "wk2": np.ascontiguousarray(inputs["Wk2"][:, s128]).astype(bf),
            "wv2": np.ascontiguousarray(inputs["Wv2"][:, s64]).astype(bf),
            "wout": np.ascontiguousarray(inputs["Wout"][s64, :]).astype(bf),
            "sink": np.ascontiguousarray(
                inputs["attn_sink"][g * 4:(g + 1) * 4]).reshape(1, HEADS)
                .astype(np.float32),
        })
    return in_maps


def kernel(**inputs):
    from concourse.bass_utils import run_bass_kernel_spmd

    inputs = {k: np.asarray(v) for k, v in inputs.items()}
    nc, _ = build_nc()
    in_maps = make_in_maps(inputs)
    res = run_bass_kernel_spmd(nc, in_maps, list(range(N_CORES)))
    out = np.zeros((2, N, DQ), dtype=np.float32)
    for c in range(N_CORES):
        out[c // 4] += res.results[c]["out"]
    return out
